# revision 1
# baseline (speedup 1.0000x reference)
"""HGAT (2-layer hyperbolic graph attention) Trainium2 kernel, 8-core SPMD.

Sharding: nodes (rows of x/adj) split 8 ways. Per layer the [N,128] tangent
features are all-gathered; softmax rows are local.

Attention decomposition (exact): with s_ij = el_i + er_j,
  exp(leaky_relu(s)) = 1{s>0} e^{el_i} e^{er_j} + 1{s<=0} e^{.2 el_i} e^{.2 er_j}
so  att-weighted agg = [u+ . (A+ @ v+) + u- . (A- @ v-)] row-normalized, where
  A+ = adj * 1{s>0},  A- = adj * 1{s<=0}  (bf16 0/1 masks, built by one fused
  scalar_tensor_tensor compare against a resident threshold matrix
  thrT = 256*(1-adj^T)), and v+/- = [e^{er} * xt | e^{er}] etc.
All transcendentals run on [N]-sized vectors only; the [N/8, N] score matrix
is touched by exactly two DVE compare ops + two bf16 matmuls per 128-chunk.
"""
import sys
import numpy as np

sys.path.insert(0, "/opt/trn_rl_repo")
sys.path.insert(0, "/opt/trn_rl_repo/concourse")

import ml_dtypes
from contextlib import ExitStack

import concourse.bass as bass
import concourse.tile as tile
from concourse import bacc, mybir
from concourse import bass_utils

F32 = mybir.dt.float32
BF16 = mybir.dt.bfloat16
AF = mybir.ActivationFunctionType
OP = mybir.AluOpType

N = 8192
D = 128
NC = 8
R = N // NC          # 1024 local rows
NT = R // 128        # 8 row tiles
NCHUNK = N // 128    # 64 j-chunks
MIN = 1e-15
ATANH_CLIP = 1.0 - 1e-5
MAXN = 1.0 - 4e-3
BIG = 256.0
GW = 132             # gather row width (bf16): 128 xt + 1 one + 1 pad + 2 (er as f32)


def _sqrt(nc, pool, out, in2):
    """out = max(sqrt(in2), MIN) via exp(0.5*ln(x)) — keeps ACT on the single
    {Ln, Exp} table set (the Sqrt table is loose and lives in another set)."""
    p, k = in2.shape
    ln = pool.tile([p, k], F32, tag="sq_ln")
    nc.scalar.activation(ln[:], in2[:], AF.Ln)
    y = pool.tile([p, k], F32, tag="sq_y")
    nc.scalar.activation(y[:], ln[:], AF.Exp, scale=0.5)
    nc.vector.tensor_scalar(out[:], y[:], MIN, None, OP.max)


def _tanh(nc, pool, out, x):
    """out = tanh(x) = 1 - 2/(exp(2x)+1), x >= 0 — Exp-table only."""
    p, k = x.shape
    e = pool.tile([p, k], F32, tag="th_e")
    nc.scalar.activation(e[:], x[:], AF.Exp, scale=2.0)
    d = pool.tile([p, k], F32, tag="th_d")
    nc.vector.tensor_scalar(d[:], e[:], 1.0, None, OP.add)
    r = pool.tile([p, k], F32, tag="th_r")
    nc.vector.reciprocal(r[:], d[:])
    nc.vector.tensor_scalar(out[:], r[:], -2.0, 1.0, OP.mult, OP.add)


def _atanh(nc, pool, out, c):
    """out = atanh(c) = 0.5*ln((1+c)/(1-c)); c in [0, 1-1e-5]. [p, k]."""
    p, k = c.shape
    ap1 = pool.tile([p, k], F32, tag="at_ap")
    nc.vector.tensor_scalar(ap1[:], c[:], 1.0, None, OP.add)
    am1 = pool.tile([p, k], F32, tag="at_am")
    nc.vector.tensor_scalar(am1[:], c[:], -1.0, 1.0, OP.mult, OP.add)
    inv = pool.tile([p, k], F32, tag="at_inv")
    nc.vector.reciprocal(inv[:], am1[:])
    ratio = pool.tile([p, k], F32, tag="at_ratio")
    nc.vector.tensor_tensor(ratio[:], ap1[:], inv[:], OP.mult)
    ln = pool.tile([p, k], F32, tag="at_ln")
    nc.scalar.activation(ln[:], ratio[:], AF.Ln)
    nc.vector.tensor_scalar(out[:], ln[:], 0.5, None, OP.mult)


def _dot_cols(nc, pool, out_col, a, b_t):
    """out_col [p,1] = sum over free dim of a*b_t, on DVE.
    (tensor_tensor_reduce crashes the runtime; use STT with accum_out.)"""
    p = a.shape[0]
    k = int(np.prod(a.shape[1:]))
    scratch = pool.tile([p, k], F32, tag="dot_scr")
    nc.vector.scalar_tensor_tensor(
        scratch[:], a[:], 1.0, b_t[:], OP.mult, OP.mult, accum_out=out_col[:]
    )


def _dot_self(nc, pool, out_col, a):
    """out_col [p,1] = sum(a*a) along free dim — on ACT (Square + accumulate),
    freeing DVE."""
    p = a.shape[0]
    k = int(np.prod(a.shape[1:]))
    scratch = pool.tile([p, k], F32, tag="dot_scr2")
    nc.scalar.activation(scratch[:], a[:], AF.Square, accum_out=out_col[:])


def build_program():
    import os
    stop = int(os.environ.get("HGAT_STOP", "9"))
    nc = bacc.Bacc(
        "TRN2", target_bir_lowering=False, debug=False, num_devices=NC
    )
    # raise the tile allocator SBUF cap (stale 192KB constant; trn2 has 208KB usable)
    try:
        from concourse import tile_utils
        tile_utils.max_sbuf_usage = 206 * 1024
    except Exception:
        pass
    try:
        tile.max_sbuf_usage = 206 * 1024
    except Exception:
        pass

    x_in = nc.dram_tensor("x_shard", [R, D], F32, kind="ExternalInput").ap()
    thr_in = nc.dram_tensor("thr_shard", [N, R], BF16, kind="ExternalInput").ap()
    wts = {}
    for li in (1, 2):
        wts[f"WT{li}"] = nc.dram_tensor(f"WT{li}", [D, D], F32, kind="ExternalInput").ap()
        for v in ("b", "al", "ar"):
            wts[f"{v}{li}"] = nc.dram_tensor(f"{v}{li}", [1, D], F32, kind="ExternalInput").ap()
    ident_f32 = nc.dram_tensor("ident_f32", [128, 128], F32, kind="ExternalInput").ap()
    y_out = nc.dram_tensor("y_shard", [R, D], F32, kind="ExternalOutput").ap()

    with tile.TileContext(nc) as tc, ExitStack() as ctx:
        # ---------------- pools ----------------
        big = ctx.enter_context(tc.tile_pool(name="big", bufs=1))      # residents
        sp = ctx.enter_context(tc.tile_pool(name="scal", bufs=1))      # small scalar tiles
        work = ctx.enter_context(tc.tile_pool(name="work", bufs=1))    # [128,128] f32 work
        dram = ctx.enter_context(tc.tile_pool(name="dram", bufs=1, space="DRAM"))

        # ---------------- constants ----------------
        idf = big.tile([128, 128], F32, tag="idf")
        nc.sync.dma_start(idf[:], ident_f32[:])

        # ------- thr = 256*(1-adj^T) bf16, host-prepped, loaded resident -------
        # thr_all[p, c, i] = 256*(1 - adj[i_global, c*128+p])
        thr_all = big.tile([128, NCHUNK, R], BF16, tag="thr")
        for q in range(4):
            nc.sync.dma_start(
                thr_all[:, q * 16:(q + 1) * 16, :],
                thr_in[q * 16 * 128:(q + 1) * 16 * 128, :]
                .rearrange("(c p) i -> p c i", p=128),
            )

        stage = ctx.enter_context(tc.tile_pool(name="stage", bufs=1))
        mpool = ctx.enter_context(tc.tile_pool(name="masks", bufs=2))

        def early_out(tiles):
            for tt_, tl in enumerate(tiles):
                cp = work.tile([128, D], F32, tag="eo", bufs=2, name=f"eo{tt_}")
                nc.vector.tensor_copy(cp[:], tl[:] if tl.shape[-1] == D else tl[:, 0:D])
                nc.sync.dma_start(y_out[tt_ * 128:(tt_ + 1) * 128, :], cp[:])

        if stop == 1:
            early_out([thr_all[:, c, 0:D] for c in range(NT)])

        # ---------------- encode: p = proj(expmap0(x)) ----------------
        run_layers = stop >= 2
        p_tiles = []
        if stop >= 2:
            n2 = sp.tile([128, NT], F32, tag="enc_n2")
            xs = []
            for t in range(NT):
                xt_ = work.tile([128, D], F32, tag=f"px{t}")
                nc.sync.dma_start(xt_[:], x_in[t * 128:(t + 1) * 128, :])
                xs.append(xt_)
                _dot_self(nc, sp, n2[:, t:t + 1], xt_)
            nrm = sp.tile([128, NT], F32, tag="enc_n")
            _sqrt(nc, sp, nrm, n2)
            th = sp.tile([128, NT], F32, tag="enc_th")
            _tanh(nc, sp, th, nrm)
            thc = sp.tile([128, NT], F32, tag="enc_thc")
            nc.vector.tensor_scalar(thc[:], th[:], MAXN, None, OP.min)
            invn = sp.tile([128, NT], F32, tag="enc_invn")
            nc.vector.reciprocal(invn[:], nrm[:])
            sc = sp.tile([128, NT], F32, tag="enc_sc")
            nc.vector.tensor_tensor(sc[:], thc[:], invn[:], OP.mult)
            for t in range(NT):
                nc.vector.tensor_scalar(
                    xs[t][:], xs[t][:], sc[:, t:t + 1], None, OP.mult
                )
            p_tiles = xs

        if stop == 2:
            early_out(p_tiles)
            run_layers = False

        # ---------------- two HGAT layers ----------------
        for li in (1, 2) if run_layers else ():
            WT_sb = big.tile([128, 128], F32, tag="WT")
            nc.sync.dma_start(WT_sb[:], wts[f"WT{li}"][:])
            b_sb = sp.tile([1, D], F32, tag="b_sb")
            nc.sync.dma_start(b_sb[:], wts[f"b{li}"][:])
            al_sb = sp.tile([1, D], F32, tag="al_sb")
            nc.sync.dma_start(al_sb[:], wts[f"al{li}"][:])
            ar_sb = sp.tile([1, D], F32, tag="ar_sb")
            nc.sync.dma_start(ar_sb[:], wts[f"ar{li}"][:])

            # ---- mobius_matvec: h = proj(tanh(mxn/xn*atanh(clip(xn))) mx/mxn)
            xn2 = sp.tile([128, NT], F32, tag="xn2")
            for t in range(NT):
                _dot_self(nc, sp, xn2[:, t:t + 1], p_tiles[t])
            xn = sp.tile([128, NT], F32, tag="xn")
            _sqrt(nc, sp, xn, xn2)
            mx_tiles = []
            mxn2 = sp.tile([128, NT], F32, tag="mxn2")
            with tc.tile_pool(name=f"ps_w{li}", bufs=2, space="PSUM") as psw:
                for t in range(NT):
                    ptp = psw.tile([128, 128], F32, tag="ptp")
                    nc.tensor.transpose(ptp[:], p_tiles[t][:], idf[:])
                    pT = work.tile([128, 128], F32, tag="pT", bufs=2)
                    nc.vector.tensor_copy(pT[:], ptp[:])
                    mxp = psw.tile([128, 128], F32, tag="mxp")
                    nc.tensor.matmul(mxp[:], pT[:], WT_sb[:], start=True, stop=True)
                    mx_sb = work.tile([128, 128], F32, tag=f"mx{t}", name=f"mx{t}")
                    nc.vector.tensor_copy(mx_sb[:], mxp[:])
                    mx_tiles.append(mx_sb)
                    _dot_self(nc, sp, mxn2[:, t:t + 1], mx_sb)
            mxn = sp.tile([128, NT], F32, tag="mxn")
            _sqrt(nc, sp, mxn, mxn2)
            cx = sp.tile([128, NT], F32, tag="cx")
            nc.vector.tensor_scalar(cx[:], xn[:], ATANH_CLIP, None, OP.min)
            at = sp.tile([128, NT], F32, tag="at")
            _atanh(nc, sp, at, cx)
            ixn = sp.tile([128, NT], F32, tag="ixn")
            nc.vector.reciprocal(ixn[:], xn[:])
            q = sp.tile([128, NT], F32, tag="q")
            nc.vector.tensor_tensor(q[:], at[:], ixn[:], OP.mult)
            arg = sp.tile([128, NT], F32, tag="arg")
            nc.vector.tensor_tensor(arg[:], q[:], mxn[:], OP.mult)
            thm = sp.tile([128, NT], F32, tag="thm")
            _tanh(nc, sp, thm, arg)
            imxn = sp.tile([128, NT], F32, tag="imxn")
            nc.vector.reciprocal(imxn[:], mxn[:])
            # proj scale on result (|res| = thm): min(thm, MAXN)
            thmc = sp.tile([128, NT], F32, tag="thmc")
            nc.vector.tensor_scalar(thmc[:], thm[:], MAXN, None, OP.min)
            lam = sp.tile([128, NT], F32, tag="lam")
            nc.vector.tensor_tensor(lam[:], thmc[:], imxn[:], OP.mult)
            for t in range(NT):
                nc.scalar.activation(
                    mx_tiles[t][:], mx_tiles[t][:], AF.Copy, scale=lam[:, t:t + 1]
                )
            h_tiles = mx_tiles

            # ---- bh = proj(expmap0(b)), broadcast ----
            bn2 = sp.tile([1, 1], F32, tag="bn2")
            _dot_self(nc, sp, bn2, b_sb)
            bn = sp.tile([1, 1], F32, tag="bn")
            _sqrt(nc, sp, bn, bn2)
            bth = sp.tile([1, 1], F32, tag="bth")
            _tanh(nc, sp, bth, bn)
            bthc = sp.tile([1, 1], F32, tag="bthc")
            nc.vector.tensor_scalar(bthc[:], bth[:], MAXN, None, OP.min)
            ibn = sp.tile([1, 1], F32, tag="ibn")
            nc.vector.reciprocal(ibn[:], bn[:])
            bsc = sp.tile([1, 1], F32, tag="bsc")
            nc.vector.tensor_tensor(bsc[:], bthc[:], ibn[:], OP.mult)
            bh = sp.tile([1, D], F32, tag="bh")
            nc.vector.tensor_scalar(bh[:], b_sb[:], bsc[:], None, OP.mult)
            bh_b = big.tile([128, D], F32, tag="bh_b")
            nc.gpsimd.partition_broadcast(bh_b[:], bh[:])
            bh2 = sp.tile([1, 1], F32, tag="bh2")
            _dot_self(nc, sp, bh2, bh)
            bh2_b = sp.tile([128, 1], F32, tag="bh2_b")
            nc.gpsimd.partition_broadcast(bh2_b[:], bh2[:])

            # ---- mobius_add(h, bh) + proj ----
            h2_tiles = []
            hn = sp.tile([128, NT], F32, tag="hn")
            hn2s = sp.tile([128, NT], F32, tag="hn2s")
            for t in range(NT):
                x2 = sp.tile([128, 1], F32, tag="x2")
                _dot_self(nc, sp, x2, h_tiles[t])
                xy = sp.tile([128, 1], F32, tag="xy")
                _dot_cols(nc, sp, xy, h_tiles[t], bh_b)
                # cf = 1 + 2xy + y2 ; den = 1 + 2xy + x2*y2
                cf = sp.tile([128, 1], F32, tag="cf")
                nc.vector.scalar_tensor_tensor(cf[:], xy[:], 2.0, bh2_b[:], OP.mult, OP.add)
                nc.vector.tensor_scalar(cf[:], cf[:], 1.0, None, OP.add)
                x2y2 = sp.tile([128, 1], F32, tag="x2y2")
                nc.vector.tensor_tensor(x2y2[:], x2[:], bh2_b[:], OP.mult)
                den = sp.tile([128, 1], F32, tag="den")
                nc.vector.scalar_tensor_tensor(den[:], xy[:], 2.0, x2y2[:], OP.mult, OP.add)
                nc.vector.tensor_scalar(den[:], den[:], 1.0, MIN, OP.add, OP.max)
                iden = sp.tile([128, 1], F32, tag="iden")
                nc.vector.reciprocal(iden[:], den[:])
                cb = sp.tile([128, 1], F32, tag="cb")
                nc.vector.tensor_scalar(cb[:], x2[:], -1.0, 1.0, OP.mult, OP.add)
                na = work.tile([128, D], F32, tag="na", bufs=2)
                nc.vector.tensor_scalar(na[:], h_tiles[t][:], cf[:], None, OP.mult)
                nb = work.tile([128, D], F32, tag="nb", bufs=2)
                nc.vector.tensor_scalar(nb[:], bh_b[:], cb[:], None, OP.mult)
                nc.vector.tensor_tensor(na[:], na[:], nb[:], OP.add)
                h2 = work.tile([128, D], F32, tag=f"h2_{t}")
                nc.vector.tensor_scalar(h2[:], na[:], iden[:], None, OP.mult)
                h2_tiles.append(h2)
                _dot_self(nc, sp, hn2s[:, t:t + 1], h2)
            _sqrt(nc, sp, hn, hn2s)
            ihn = sp.tile([128, NT], F32, tag="ihn")
            nc.vector.reciprocal(ihn[:], hn[:])
            psc = sp.tile([128, NT], F32, tag="psc")
            nc.vector.tensor_scalar(psc[:], ihn[:], MAXN, 1.0, OP.mult, OP.min)
            # h3 = proj(h2); xt = atanh(clip(|h3|))/|h3| * h3
            # |h3| = min(hn, MAXN); xt = h2 * psc * atanh(clip(min(hn,MAXN)))/min(hn,MAXN)
            hnc = sp.tile([128, NT], F32, tag="hnc")
            nc.vector.tensor_scalar(hnc[:], hn[:], MAXN, ATANH_CLIP, OP.min, OP.min)
            ath = sp.tile([128, NT], F32, tag="ath")
            _atanh(nc, sp, ath, hnc)
            # xt = h2 * (psc * ath / min(hn, MAXN)) ; note psc/min(hn,MAXN) = min(1/hn, ...)
            hm = sp.tile([128, NT], F32, tag="hm")
            nc.vector.tensor_scalar(hm[:], hn[:], MAXN, None, OP.min)
            ihm = sp.tile([128, NT], F32, tag="ihm")
            nc.vector.reciprocal(ihm[:], hm[:])
            xsc = sp.tile([128, NT], F32, tag="xsc")
            nc.vector.tensor_tensor(xsc[:], psc[:], ihm[:], OP.mult)
            nc.vector.tensor_tensor(xsc[:], xsc[:], ath[:], OP.mult)
            for t in range(NT):
                nc.scalar.activation(
                    h2_tiles[t][:], h2_tiles[t][:], AF.Copy, scale=xsc[:, t:t + 1]
                )
            xt_tiles = h2_tiles

            if stop == 3 and li == 1:
                early_out(xt_tiles)
                break

            # ---- el, er_local ----
            al_b = big.tile([128, D], F32, tag="al_b")
            nc.gpsimd.partition_broadcast(al_b[:], al_sb[:])
            ar_b = big.tile([128, D], F32, tag="ar_b")
            nc.gpsimd.partition_broadcast(ar_b[:], ar_sb[:])
            el = sp.tile([128, NT], F32, tag="el")
            erl = sp.tile([128, NT], F32, tag="erl")
            for t in range(NT):
                _dot_cols(nc, sp, el[:, t:t + 1], xt_tiles[t], al_b)
                _dot_cols(nc, sp, erl[:, t:t + 1], xt_tiles[t], ar_b)

            # ---- build + AllGather [xt_bf16 | 1 | pad | er_f32] ----
            send = dram.tile([R, GW], BF16, tag=f"send{li}")
            gath = dram.tile([N, GW], BF16, tag=f"gath{li}", addr_space="Shared")
            for t in range(NT):
                sb_send = stage.tile([128, GW], BF16, tag="sb_send", bufs=2)
                nc.vector.tensor_copy(sb_send[:, 0:D], xt_tiles[t][:])
                nc.vector.memset(sb_send[:, D:D + 2], 0.0)
                nc.vector.memset(sb_send[:, D:D + 1], 1.0)
                nc.vector.tensor_copy(
                    sb_send[:].bitcast(F32)[:, 65:66], erl[:, t:t + 1]
                )
                nc.sync.dma_start(send[t * 128:(t + 1) * 128, :], sb_send[:])
            nc.gpsimd.collective_compute(
                "AllGather",
                OP.bypass,
                replica_groups=[list(range(NC))],
                ins=[send[:]],
                outs=[gath[:]],
            )

            # ---- er over all nodes; exp factors ----
            gath_f32 = gath[:].bitcast(F32)  # [N, 66]
            er_all = sp.tile([128, NCHUNK], F32, tag="er_all")
            nc.sync.dma_start(
                er_all[:],
                gath_f32[:, 65:66].rearrange("(c p) one -> p (c one)", p=128),
            )
            wp = sp.tile([128, NCHUNK], F32, tag="wp")
            nc.scalar.activation(wp[:], er_all[:], AF.Exp)
            wm = sp.tile([128, NCHUNK], F32, tag="wm")
            nc.scalar.activation(wm[:], er_all[:], AF.Exp, scale=0.2)
            up = sp.tile([128, NT], F32, tag="up")
            nc.scalar.activation(up[:], el[:], AF.Exp)
            um = sp.tile([128, NT], F32, tag="um")
            nc.scalar.activation(um[:], el[:], AF.Exp, scale=0.2)

            # ---- el broadcast along free dim (via DRAM round-trip) ----
            el_dram = dram.tile([1, R], F32, tag=f"eld{li}")
            nc.sync.dma_start(
                el_dram[:].rearrange("one (t p) -> (one p) t", p=128), el[:]
            )
            el_row = stage.tile([1, R], BF16, tag="el_row")
            nc.gpsimd.dma_start(el_row[:], el_dram[:])  # cast f32->bf16
            el_b = big.tile([128, R], BF16, tag="el_b")
            nc.gpsimd.partition_broadcast(el_b[:], el_row[:])

            # ---- v = [v+ | v-] resident; CS = colsum(v-) via ones-matmul ----
            W2C = 2 * (D + 1)  # 258
            vall = big.tile([128, NCHUNK, W2C], BF16, tag="vall")
            ones_bf = sp.tile([128, 1], BF16, tag="ones_bf")
            nc.vector.memset(ones_bf[:], 1.0)
            with tc.tile_pool(name=f"xg{li}", bufs=1) as xgp, \
                    tc.tile_pool(name=f"pcs{li}", bufs=1, space="PSUM") as pcs:
                cs_ps = pcs.tile([1, D + 1], F32, tag="cs")
                for gblk in range(4):
                    xtg = xgp.tile([128, 16, D + 1], BF16, tag="xtg")
                    nc.sync.dma_start(
                        xtg[:],
                        gath[gblk * 16 * 128:(gblk + 1) * 16 * 128, 0:D + 1]
                        .rearrange("(c p) w -> p c w", p=128),
                    )
                    for k in range(16):
                        c = gblk * 16 + k
                        nc.vector.tensor_scalar(
                            vall[:, c, 0:D + 1], xtg[:, k, :], wp[:, c:c + 1],
                            None, OP.mult,
                        )
                        nc.vector.tensor_scalar(
                            vall[:, c, D + 1:W2C], xtg[:, k, :], wm[:, c:c + 1],
                            None, OP.mult,
                        )
                        nc.tensor.matmul(
                            cs_ps[:], ones_bf[:], vall[:, c, D + 1:W2C],
                            start=(c == 0), stop=(c == NCHUNK - 1),
                        )
                cs_sb = sp.tile([1, D + 1], F32, tag="cs_sb")
                nc.vector.tensor_copy(cs_sb[:], cs_ps[:])
            csb = big.tile([128, D + 1], F32, tag="csb")
            nc.gpsimd.partition_broadcast(csb[:], cs_sb[:])

            if stop == 4 and li == 1:
                early_out([vall[:, c, 0:D] for c in range(NT)])
                break

            # ---- masked attention matmuls: 3 streams, A+ mask only ----
            # acc[t] = [ A+@v+ (0:129) | A+@v- (129:258) | thr@v- (258:387) ]
            psa = tc.alloc_tile_pool(name=f"ps_acc{li}", bufs=1, space="PSUM")
            acc = [
                psa.tile([128, 3 * (D + 1)], F32, tag=f"acc{t}", name=f"acc{t}_l{li}")
                for t in range(NT)
            ]
            for c in range(NCHUNK):
                s_c = mpool.tile([128, R], BF16, tag="sc")
                nc.vector.tensor_scalar(
                    s_c[:], el_b[:], er_all[:, c:c + 1], None, OP.add
                )
                ap_ = mpool.tile([128, R], BF16, tag="Ap")
                nc.vector.tensor_tensor(ap_[:], s_c[:], thr_all[:, c, :], OP.is_gt)
                # One accumulation group per PSUM bank: start only on the very
                # first matmul into the bank (pending-zero covers the whole
                # bank), stop only on the very last.
                first = c == 0
                last = c == NCHUNK - 1
                for t in range(NT):
                    nc.tensor.matmul(
                        acc[t][:, 0:W2C],
                        ap_[:, t * 128:(t + 1) * 128],
                        vall[:, c, :],
                        start=first, stop=False,
                    )
                    nc.tensor.matmul(
                        acc[t][:, W2C:W2C + D + 1],
                        thr_all[:, c, t * 128:(t + 1) * 128],
                        vall[:, c, D + 1:W2C],
                        start=False, stop=last,
                    )

            # ---- evict + normalize -> agg ----
            # A-@v- = CS - thr@v-/256 - A+@v-
            agg_tiles = []
            for t in range(NT):
                e1 = work.tile([128, D + 1], F32, tag="e1")
                nc.vector.tensor_scalar(
                    e1[:], acc[t][:, W2C:W2C + D + 1], -1.0 / BIG, None, OP.mult
                )
                nc.vector.scalar_tensor_tensor(
                    e1[:], acc[t][:, D + 1:W2C], -1.0, e1[:], OP.mult, OP.add
                )
                t2 = work.tile([128, D + 1], F32, tag="t2")
                nc.vector.tensor_tensor(t2[:], e1[:], csb[:], OP.add)
                r1 = work.tile([128, D + 1], F32, tag="r1")
                nc.vector.tensor_scalar(
                    r1[:], acc[t][:, 0:D + 1], up[:, t:t + 1], None, OP.mult
                )
                res = work.tile([128, D + 1], F32, tag="res")
                nc.vector.scalar_tensor_tensor(
                    res[:], t2[:], um[:, t:t + 1], r1[:], OP.mult, OP.add,
                )
                dn = sp.tile([128, 1], F32, tag="dn")
                nc.vector.tensor_scalar(dn[:], res[:, D:D + 1], MIN, None, OP.max)
                idn = sp.tile([128, 1], F32, tag="idn")
                nc.vector.reciprocal(idn[:], dn[:])
                ag = work.tile([128, D], F32, tag=f"ag{t}")
                nc.scalar.activation(ag[:], res[:, 0:D], AF.Copy, scale=idn[:])
                agg_tiles.append(ag)
            psa.release()

            if stop == 5 and li == 1:
                early_out(agg_tiles)
                break

            # ---- tail: out = proj(expmap0(relu(logmap0(proj(expmap0(agg)))))) ----
            an2 = sp.tile([128, NT], F32, tag="an2")
            for t in range(NT):
                _dot_self(nc, sp, an2[:, t:t + 1], agg_tiles[t])
            an = sp.tile([128, NT], F32, tag="an")
            _sqrt(nc, sp, an, an2)
            ath2 = sp.tile([128, NT], F32, tag="ath2")
            _tanh(nc, sp, ath2, an)
            # n2 = min(tanh, MAXN); c3 = min(n2, CLIP); xt2 = agg * atanh(c3)/an
            c3 = sp.tile([128, NT], F32, tag="c3")
            nc.vector.tensor_scalar(c3[:], ath2[:], MAXN, ATANH_CLIP, OP.min, OP.min)
            at3 = sp.tile([128, NT], F32, tag="at3")
            _atanh(nc, sp, at3, c3)
            ian = sp.tile([128, NT], F32, tag="ian")
            nc.vector.reciprocal(ian[:], an[:])
            sc3 = sp.tile([128, NT], F32, tag="sc3")
            nc.vector.tensor_tensor(sc3[:], at3[:], ian[:], OP.mult)
            rn2 = sp.tile([128, NT], F32, tag="rn2")
            for t in range(NT):
                nc.vector.tensor_scalar(
                    agg_tiles[t][:], agg_tiles[t][:], sc3[:, t:t + 1], 0.0,
                    OP.mult, OP.max,
                )
                _dot_cols(nc, sp, rn2[:, t:t + 1], agg_tiles[t], agg_tiles[t])
            r_tiles = agg_tiles
            rn = sp.tile([128, NT], F32, tag="rn")
            _sqrt(nc, sp, rn, rn2)
            rth = sp.tile([128, NT], F32, tag="rth")
            _tanh(nc, sp, rth, rn)
            rthc = sp.tile([128, NT], F32, tag="rthc")
            nc.vector.tensor_scalar(rthc[:], rth[:], MAXN, None, OP.min)
            irn = sp.tile([128, NT], F32, tag="irn")
            nc.vector.reciprocal(irn[:], rn[:])
            fsc = sp.tile([128, NT], F32, tag="fsc")
            nc.vector.tensor_tensor(fsc[:], rthc[:], irn[:], OP.mult)
            for t in range(NT):
                nc.scalar.activation(
                    r_tiles[t][:], r_tiles[t][:], AF.Copy, scale=fsc[:, t:t + 1]
                )
            p_tiles = r_tiles
            if stop == 6 and li == 1:
                early_out(p_tiles)
                break

        # ---------------- output ----------------
        if stop >= 7:
            for t in range(NT):
                nc.sync.dma_start(y_out[t * 128:(t + 1) * 128, :], p_tiles[t][:])

    nc.compile()
    return nc


_PROGRAM = None


def kernel(**inputs):
    global _PROGRAM
    if _PROGRAM is None:
        _PROGRAM = build_program()
    nc = _PROGRAM

    x = np.ascontiguousarray(inputs["x"], np.float32)
    adj = np.asarray(inputs["adj"], np.float32)
    base = {
        "WT1": np.ascontiguousarray(inputs["W1"].T, np.float32),
        "WT2": np.ascontiguousarray(inputs["W2"].T, np.float32),
        "b1": np.asarray(inputs["b1"], np.float32).reshape(1, D),
        "b2": np.asarray(inputs["b2"], np.float32).reshape(1, D),
        "al1": np.asarray(inputs["al1"], np.float32).reshape(1, D),
        "al2": np.asarray(inputs["al2"], np.float32).reshape(1, D),
        "ar1": np.asarray(inputs["ar1"], np.float32).reshape(1, D),
        "ar2": np.asarray(inputs["ar2"], np.float32).reshape(1, D),
        "ident_f32": np.eye(128, dtype=np.float32),
    }
    in_maps = []
    for r in range(NC):
        m = dict(base)
        m["x_shard"] = x[r * R:(r + 1) * R]
        # sharded layout prep: per-core threshold matrix 256*(1-adj_rows)^T, bf16
        m["thr_shard"] = np.ascontiguousarray(
            (256.0 - 256.0 * adj[r * R:(r + 1) * R].T).astype(ml_dtypes.bfloat16)
        )
        in_maps.append(m)

    global _last_in_maps
    _last_in_maps = in_maps
    res = bass_utils.run_bass_kernel_spmd(nc, in_maps, core_ids=list(range(NC)))
    return np.concatenate([res.results[r]["y_shard"] for r in range(NC)], axis=0)


_last_in_maps = None


if __name__ == "__main__":
    import reference
    inputs = {k: np.asarray(v) for k, v in reference.setup_inputs().items()}
    out = kernel(**inputs)
    print("out", out.shape, out.dtype)



# revision 9
# speedup vs baseline: 1.1744x; 1.1744x over previous
"""HGAT (2-layer hyperbolic graph attention) Trainium2 kernel, 8-core SPMD.

Sharding: nodes (rows of x/adj) split 8 ways. Per layer the [N,132] payload
(xt bf16 | 1 | pad | er f32) is all-gathered in two row-halves so the mask
loop can start after the first half lands; softmax rows are local.

Attention decomposition (exact): with s_ij = el_i + er_j,
  exp(leaky_relu(s)) = 1{s>0} e^{el_i} e^{er_j} + 1{s<=0} e^{.2 el_i} e^{.2 er_j}
so  att-weighted agg = [u+ . (A+ @ v+) + u- . (A- @ v-)] row-normalized, where
  A+ = adj * 1{s>0} (bf16 0/1 mask, built by ONE fused scalar_tensor_tensor
  (el_b + er_j) is_gt thr against a resident threshold matrix
  thr = 256*(1-adj^T), host-prepped in processing-chunk order), and
  A-@v- = CS - (thr@v-)/256 - A+@v-  with CS = colsum(v-) computed from local
  shards and summed with a tiny AllReduce that hides under the mask loop.
All hyperbolic chains (expmap/logmap/mobius ops) are folded into per-node
column scalars; per [128,128] tile only two fused DVE ops materialize xt.
"""
import sys
import numpy as np

sys.path.insert(0, "/opt/trn_rl_repo")
sys.path.insert(0, "/opt/trn_rl_repo/concourse")

import ml_dtypes
from contextlib import ExitStack

import concourse.bass as bass
import concourse.tile as tile
from concourse import bacc, mybir
from concourse import bass_utils

F32 = mybir.dt.float32
BF16 = mybir.dt.bfloat16
AF = mybir.ActivationFunctionType
OP = mybir.AluOpType

N = 8192
D = 128
NC = 8
R = N // NC          # 1024 local rows
NT = R // 128        # 8 row tiles
NK = N // 128        # 64 j-chunks
MIN = 1e-15
ATANH_CLIP = 1.0 - 1e-5
MAXN = 1.0 - 4e-3
BIG = 256.0
GW = 132             # gather row width (bf16): 128 xt + 1 one + 1 pad + 2 (er as f32)
CSW = 132            # padded colsum width for the AllReduce

# program chunk order: first the 32 "low half" chunks (each core's local rows
# 0:512), then the 32 "high half" chunks. PO[k] = global chunk index.
PO = [8 * r + hc for r in range(8) for hc in range(4)] + \
     [8 * r + hc for r in range(8) for hc in range(4, 8)]


def _sqrt(nc, pool, out, in2, tag):
    """out = max(sqrt(in2), MIN) via exp(0.5*ln(x)) — keeps ACT on {Ln, Exp}."""
    p, k = in2.shape
    ln = pool.tile([p, k], F32, tag=tag + "_ln")
    nc.scalar.activation(ln[:], in2[:], AF.Ln)
    y = pool.tile([p, k], F32, tag=tag + "_y")
    nc.scalar.activation(y[:], ln[:], AF.Exp, scale=0.5)
    nc.vector.tensor_scalar(out[:], y[:], MIN, None, OP.max)


def _tanh(nc, pool, out, x, tag):
    """out = tanh(x) = 1 - 2/(exp(2x)+1), x >= 0 — Exp-table only."""
    p, k = x.shape
    e = pool.tile([p, k], F32, tag=tag + "_e")
    nc.scalar.activation(e[:], x[:], AF.Exp, scale=2.0)
    d = pool.tile([p, k], F32, tag=tag + "_d")
    nc.vector.tensor_scalar(d[:], e[:], 1.0, None, OP.add)
    r = pool.tile([p, k], F32, tag=tag + "_r")
    nc.vector.reciprocal(r[:], d[:])
    nc.vector.tensor_scalar(out[:], r[:], -2.0, 1.0, OP.mult, OP.add)


def _atanh(nc, pool, out, c, tag):
    """out = atanh(c) = 0.5*ln((1+c)/(1-c)); c in [0, 1-1e-5]."""
    p, k = c.shape
    ap1 = pool.tile([p, k], F32, tag=tag + "_ap")
    nc.vector.tensor_scalar(ap1[:], c[:], 1.0, None, OP.add)
    am1 = pool.tile([p, k], F32, tag=tag + "_am")
    nc.vector.tensor_scalar(am1[:], c[:], -1.0, 1.0, OP.mult, OP.add)
    inv = pool.tile([p, k], F32, tag=tag + "_inv")
    nc.vector.reciprocal(inv[:], am1[:])
    ratio = pool.tile([p, k], F32, tag=tag + "_ratio")
    nc.vector.tensor_tensor(ratio[:], ap1[:], inv[:], OP.mult)
    ln = pool.tile([p, k], F32, tag=tag + "_lg")
    nc.scalar.activation(ln[:], ratio[:], AF.Ln)
    nc.vector.tensor_scalar(out[:], ln[:], 0.5, None, OP.mult)


def _dot_cols(nc, pool, out_col, a, b_t):
    """out_col [p,1] = sum over free dim of a*b_t, on DVE STT with accum."""
    p = a.shape[0]
    k = int(np.prod(a.shape[1:]))
    scratch = pool.tile([p, k], F32, tag="dot_scr", bufs=4)
    nc.vector.scalar_tensor_tensor(
        scratch[:], a[:], 1.0, b_t[:], OP.mult, OP.mult, accum_out=out_col[:]
    )


def build_program():
    import os
    stop = int(os.environ.get("HGAT_STOP", "9"))
    nc = bacc.Bacc(
        "TRN2", target_bir_lowering=False, debug=False, num_devices=NC
    )
    try:
        from concourse import tile_utils
        tile_utils.max_sbuf_usage = 206 * 1024
    except Exception:
        pass
    try:
        tile.max_sbuf_usage = 206 * 1024
    except Exception:
        pass

    x_in = nc.dram_tensor("x_shard", [R, D], F32, kind="ExternalInput").ap()
    # thr in PROGRAM chunk order: [N, R] rows grouped as (k, p)
    thr_in = nc.dram_tensor("thr_shard", [N, R], BF16, kind="ExternalInput").ap()
    wts = {}
    for li in (1, 2):
        wts[f"WT{li}"] = nc.dram_tensor(f"WT{li}", [D, D], F32, kind="ExternalInput").ap()
        for v in ("al", "ar", "bh"):
            wts[f"{v}{li}"] = nc.dram_tensor(f"{v}{li}", [1, D], F32, kind="ExternalInput").ap()
        wts[f"bh2_{li}"] = nc.dram_tensor(f"bh2_{li}", [1, 1], F32, kind="ExternalInput").ap()
    ident_f32 = nc.dram_tensor("ident_f32", [128, 128], F32, kind="ExternalInput").ap()
    y_out = nc.dram_tensor("y_shard", [R, D], F32, kind="ExternalOutput").ap()

    with tile.TileContext(nc) as tc, ExitStack() as ctx:
        # ---------------- pools ----------------
        big = ctx.enter_context(tc.tile_pool(name="big", bufs=1))      # residents
        sp = ctx.enter_context(tc.tile_pool(name="scal", bufs=1))      # small scalar tiles
        work = ctx.enter_context(tc.tile_pool(name="work", bufs=1))    # [128,128] f32 named tiles
        stage = ctx.enter_context(tc.tile_pool(name="stage", bufs=1))
        mpool = ctx.enter_context(tc.tile_pool(name="masks", bufs=2))
        vpool = ctx.enter_context(tc.tile_pool(name="vchunk", bufs=3))
        gpool = ctx.enter_context(tc.tile_pool(name="gblk", bufs=2))
        dram = ctx.enter_context(tc.tile_pool(name="dram", bufs=1, space="DRAM"))

        # ---------------- early small loads (sync queue, first) --------------
        idf = big.tile([128, 128], F32, tag="idf")
        nc.sync.dma_start(idf[:], ident_f32[:])
        x_tiles = []
        for t in range(NT):
            xt_ = work.tile([128, D], F32, tag=f"x{t}")
            nc.sync.dma_start(xt_[:], x_in[t * 128:(t + 1) * 128, :])
            x_tiles.append(xt_)
        w_sb = {}
        for li in (1, 2):
            w_sb[f"WT{li}"] = big.tile([128, 128], F32, tag=f"WT{li}",
                                       name=f"WT{li}")
            nc.sync.dma_start(w_sb[f"WT{li}"][:], wts[f"WT{li}"][:])
            for v in ("al", "ar", "bh"):
                w_sb[f"{v}{li}"] = sp.tile([1, D], F32, tag=f"{v}{li}_sb",
                                           name=f"{v}{li}_sb")
                nc.sync.dma_start(w_sb[f"{v}{li}"][:], wts[f"{v}{li}"][:])
            w_sb[f"bh2_{li}"] = sp.tile([1, 1], F32, tag=f"bh2_{li}_sb",
                                        name=f"bh2_{li}_sb")
            nc.sync.dma_start(w_sb[f"bh2_{li}"][:], wts[f"bh2_{li}"][:])

        # ------- thr resident, loaded on the gpsimd queue, low half first ----
        # thr_all[p, k, i] = 256*(1 - adj[i_global, jnode(PO[k], p)])
        thr_all = big.tile([128, NK, R], BF16, tag="thr")
        for q in range(4):  # low half: program chunks 0..31
            nc.gpsimd.dma_start(
                thr_all[:, q * 8:(q + 1) * 8, :],
                thr_in[q * 8 * 128:(q + 1) * 8 * 128, :]
                .rearrange("(c p) i -> p c i", p=128),
            )

        def load_thr_high():
            for q in range(4, 8):  # high half: program chunks 32..63
                nc.gpsimd.dma_start(
                    thr_all[:, q * 8:(q + 1) * 8, :],
                    thr_in[q * 8 * 128:(q + 1) * 8 * 128, :]
                    .rearrange("(c p) i -> p c i", p=128),
                )

        # ---------------- broadcast constants via K=1 matmuls ----------------
        ones1 = sp.tile([1, 128], F32, tag="ones1")
        nc.vector.memset(ones1[:], 1.0)
        bcast = {}
        with tc.tile_pool(name="ps_b", bufs=2, space="PSUM") as psb:
            for li in (1, 2):
                for v in ("al", "ar", "bh"):
                    ps = psb.tile([128, 128], F32, tag="bc_ps")
                    nc.tensor.matmul(ps[:], ones1[:], w_sb[f"{v}{li}"][:],
                                     start=True, stop=True)
                    bb = big.tile([128, 128], F32, tag=f"{v}{li}_b")
                    nc.vector.tensor_copy(bb[:], ps[:])
                    bcast[f"{v}{li}"] = bb
                ps1 = psb.tile([128, 1], F32, tag="bc_ps1")
                nc.tensor.matmul(ps1[:], ones1[:], w_sb[f"bh2_{li}"][:],
                                 start=True, stop=True)
                b1 = sp.tile([128, 1], F32, tag=f"bh2_{li}_b")
                nc.vector.tensor_copy(b1[:], ps1[:])
                bcast[f"bh2_{li}"] = b1

        def early_out(tiles, width=D):
            for tt_, tl in enumerate(tiles):
                cp = work.tile([128, D], F32, tag="eo", bufs=2, name=f"eo{tt_}")
                nc.vector.tensor_copy(cp[:], tl[:] if tl.shape[-1] == width else tl[:, 0:width])
                nc.sync.dma_start(y_out[tt_ * 128:(tt_ + 1) * 128, :], cp[:])

        # ---------------- encode (folded): p = fsc*x, |p| = xn --------------
        xn2 = sp.tile([128, NT], F32, tag="enc_n2")
        for t in range(NT):
            _dot_cols(nc, sp, xn2[:, t:t + 1], x_tiles[t], x_tiles[t])
        xnr = sp.tile([128, NT], F32, tag="enc_nr")
        _sqrt(nc, sp, xnr, xn2, "enc_sq")
        th0 = sp.tile([128, NT], F32, tag="enc_th")
        _tanh(nc, sp, th0, xnr, "enc_tanh")
        xn_enc = sp.tile([128, NT], F32, tag="enc_xn")
        nc.vector.tensor_scalar(xn_enc[:], th0[:], MAXN, MIN, OP.min, OP.max)
        ixnr = sp.tile([128, NT], F32, tag="enc_ixnr")
        nc.vector.reciprocal(ixnr[:], xnr[:])
        fsc_enc = sp.tile([128, NT], F32, tag="enc_fsc")
        nc.vector.tensor_tensor(fsc_enc[:], xn_enc[:], ixnr[:], OP.mult)

        if stop == 2:
            early_out(x_tiles)

        # ---------------- layer ----------------
        def layer(ft, fsc, xn, li):
            """Input: hyperbolic p = fsc*ft (per-node col scalar), |p| = xn.
            xt overwrites ft; returns (rf, fsc', xn') for the next layer."""
            L = f"l{li}"
            al_b, ar_b, bh_b = bcast[f"al{li}"], bcast[f"ar{li}"], bcast[f"bh{li}"]
            bh2_b = bcast[f"bh2_{li}"]
            WT_sb = w_sb[f"WT{li}"]

            # ---- matvec: fw = ft @ WT ----
            fw = []
            with tc.tile_pool(name=f"ps_w{L}", bufs=2, space="PSUM") as psw:
                for t in range(NT):
                    ptp = psw.tile([128, 128], F32, tag="ptp")
                    nc.tensor.transpose(ptp[:], ft[t][:], idf[:])
                    pT = work.tile([128, 128], F32, tag="pT", bufs=2)
                    nc.vector.tensor_copy(pT[:], ptp[:])
                    mxp = psw.tile([128, 128], F32, tag="mxp")
                    nc.tensor.matmul(mxp[:], pT[:], WT_sb[:], start=True, stop=True)
                    fw_t = work.tile([128, 128], F32, tag=f"fw{L}_{t}")
                    nc.vector.tensor_copy(fw_t[:], mxp[:])
                    fw.append(fw_t)

            fwn2 = sp.tile([128, NT], F32, tag=f"fwn2{L}")
            for t in range(NT):
                _dot_cols(nc, sp, fwn2[:, t:t + 1], fw[t], fw[t])
            fwn = sp.tile([128, NT], F32, tag=f"fwn{L}")
            _sqrt(nc, sp, fwn, fwn2, f"fwn_sq{L}")

            # ---- mobius_matvec scalars: h = lam*fw, |h| = thmc ----
            cx = sp.tile([128, NT], F32, tag=f"cx{L}")
            nc.vector.tensor_scalar(cx[:], xn[:], ATANH_CLIP, None, OP.min)
            at = sp.tile([128, NT], F32, tag=f"at{L}")
            _atanh(nc, sp, at, cx, f"at{L}")
            ixn = sp.tile([128, NT], F32, tag=f"ixn{L}")
            nc.vector.reciprocal(ixn[:], xn[:])
            qq = sp.tile([128, NT], F32, tag=f"qq{L}")
            nc.vector.tensor_tensor(qq[:], at[:], ixn[:], OP.mult)
            mxn = sp.tile([128, NT], F32, tag=f"mxn{L}")
            nc.vector.tensor_tensor(mxn[:], fsc[:], fwn[:], OP.mult)
            arg = sp.tile([128, NT], F32, tag=f"arg{L}")
            nc.vector.tensor_tensor(arg[:], qq[:], mxn[:], OP.mult)
            thm = sp.tile([128, NT], F32, tag=f"thm{L}")
            _tanh(nc, sp, thm, arg, f"thm{L}")
            thmc = sp.tile([128, NT], F32, tag=f"thmc{L}")
            nc.vector.tensor_scalar(thmc[:], thm[:], MAXN, None, OP.min)
            ifwn = sp.tile([128, NT], F32, tag=f"ifwn{L}")
            nc.vector.reciprocal(ifwn[:], fwn[:])
            lam = sp.tile([128, NT], F32, tag=f"lam{L}")
            nc.vector.tensor_tensor(lam[:], thmc[:], ifwn[:], OP.mult)

            # ---- mobius_add + proj + logmap0 folded: xt = a'*fw + beta*bh ----
            x2 = sp.tile([128, NT], F32, tag=f"x2{L}")
            nc.vector.tensor_tensor(x2[:], thmc[:], thmc[:], OP.mult)
            xyr = sp.tile([128, NT], F32, tag=f"xyr{L}")
            for t in range(NT):
                _dot_cols(nc, sp, xyr[:, t:t + 1], fw[t], bh_b)
            xy = sp.tile([128, NT], F32, tag=f"xy{L}")
            nc.vector.tensor_tensor(xy[:], lam[:], xyr[:], OP.mult)
            # cf = 1 + 2xy + y2 ; cb = 1 - x2 ; den = max(1+2xy+x2*y2, MIN)
            cf = sp.tile([128, NT], F32, tag=f"cf{L}")
            nc.vector.tensor_scalar(cf[:], xy[:], 2.0, 1.0, OP.mult, OP.add)
            nc.vector.tensor_scalar(cf[:], cf[:], bh2_b[:], None, OP.add)
            cb = sp.tile([128, NT], F32, tag=f"cb{L}")
            nc.vector.tensor_scalar(cb[:], x2[:], -1.0, 1.0, OP.mult, OP.add)
            x2y2 = sp.tile([128, NT], F32, tag=f"x2y2{L}")
            nc.vector.tensor_scalar(x2y2[:], x2[:], bh2_b[:], None, OP.mult)
            den = sp.tile([128, NT], F32, tag=f"den{L}")
            nc.vector.scalar_tensor_tensor(den[:], xy[:], 2.0, x2y2[:], OP.mult, OP.add)
            nc.vector.tensor_scalar(den[:], den[:], 1.0, MIN, OP.add, OP.max)
            # nn2 = cf^2 x2 + 2 cf cb xy + cb^2 y2
            t1 = sp.tile([128, NT], F32, tag=f"t1{L}")
            nc.vector.tensor_tensor(t1[:], cf[:], cf[:], OP.mult)
            nc.vector.tensor_tensor(t1[:], t1[:], x2[:], OP.mult)
            t2 = sp.tile([128, NT], F32, tag=f"t2{L}")
            nc.vector.tensor_tensor(t2[:], cf[:], cb[:], OP.mult)
            nc.vector.tensor_tensor(t2[:], t2[:], xy[:], OP.mult)
            nc.vector.scalar_tensor_tensor(t1[:], t2[:], 2.0, t1[:], OP.mult, OP.add)
            t3 = sp.tile([128, NT], F32, tag=f"t3{L}")
            nc.vector.tensor_tensor(t3[:], cb[:], cb[:], OP.mult)
            nc.vector.tensor_scalar(t3[:], t3[:], bh2_b[:], None, OP.mult)
            nn2 = sp.tile([128, NT], F32, tag=f"nn2{L}")
            nc.vector.tensor_tensor(nn2[:], t1[:], t3[:], OP.add)
            nn = sp.tile([128, NT], F32, tag=f"nn{L}")
            _sqrt(nc, sp, nn, nn2, f"nn_sq{L}")
            iden = sp.tile([128, NT], F32, tag=f"iden{L}")
            nc.vector.reciprocal(iden[:], den[:])
            hn = sp.tile([128, NT], F32, tag=f"hn{L}")
            nc.vector.tensor_tensor(hn[:], nn[:], iden[:], OP.mult)
            hm = sp.tile([128, NT], F32, tag=f"hm{L}")
            nc.vector.tensor_scalar(hm[:], hn[:], MAXN, None, OP.min)
            hmc = sp.tile([128, NT], F32, tag=f"hmc{L}")
            nc.vector.tensor_scalar(hmc[:], hm[:], ATANH_CLIP, None, OP.min)
            ath = sp.tile([128, NT], F32, tag=f"ath{L}")
            _atanh(nc, sp, ath, hmc, f"ath{L}")
            ihm = sp.tile([128, NT], F32, tag=f"ihm{L}")
            nc.vector.reciprocal(ihm[:], hm[:])
            ihn = sp.tile([128, NT], F32, tag=f"ihn{L}")
            nc.vector.reciprocal(ihn[:], hn[:])
            pr = sp.tile([128, NT], F32, tag=f"pr{L}")
            nc.vector.tensor_scalar(pr[:], ihn[:], MAXN, 1.0, OP.mult, OP.min)
            psi = sp.tile([128, NT], F32, tag=f"psi{L}")
            nc.vector.tensor_tensor(psi[:], ath[:], ihm[:], OP.mult)
            nc.vector.tensor_tensor(psi[:], psi[:], pr[:], OP.mult)
            nc.vector.tensor_tensor(psi[:], psi[:], iden[:], OP.mult)
            alp = sp.tile([128, NT], F32, tag=f"alp{L}")
            nc.vector.tensor_tensor(alp[:], psi[:], cf[:], OP.mult)
            nc.vector.tensor_tensor(alp[:], alp[:], lam[:], OP.mult)
            bet = sp.tile([128, NT], F32, tag=f"bet{L}")
            nc.vector.tensor_tensor(bet[:], psi[:], cb[:], OP.mult)

            xt = ft  # xt overwrites the input tiles (ft only read by transpose)
            for t in range(NT):
                tmp = work.tile([128, 128], F32, tag="xtmp", bufs=2)
                nc.vector.tensor_scalar(tmp[:], fw[t][:], alp[:, t:t + 1], None, OP.mult)
                nc.vector.scalar_tensor_tensor(
                    xt[t][:], bh_b[:], bet[:, t:t + 1], tmp[:], OP.mult, OP.add
                )

            if stop == 3 and li == 1:
                early_out(xt)
                return None

            # ---- el, er_local ----
            el = sp.tile([128, NT], F32, tag=f"el{L}")
            erl = sp.tile([128, NT], F32, tag=f"erl{L}")
            for t in range(NT):
                _dot_cols(nc, sp, el[:, t:t + 1], xt[t], al_b)
                _dot_cols(nc, sp, erl[:, t:t + 1], xt[t], ar_b)

            # ---- send build + split AllGather ----
            send = dram.tile([R, GW], BF16, tag=f"send{L}")
            gathA = dram.tile([N // 2, GW], BF16, tag=f"gathA{L}", addr_space="Shared")
            gathB = dram.tile([N // 2, GW], BF16, tag=f"gathB{L}", addr_space="Shared")
            sends = []
            for t in range(NT):
                sb_send = stage.tile([128, GW], BF16, tag=f"sb_send{t}",
                                     name=f"sb_send{L}_{t}")
                sends.append(sb_send)
                nc.vector.memset(sb_send[:, D:D + 2], 0.0)
                nc.vector.memset(sb_send[:, D:D + 1], 1.0)
            for t in range(NT):
                nc.vector.tensor_copy(sends[t][:, 0:D], xt[t][:])
                nc.vector.tensor_copy(
                    sends[t][:].bitcast(F32)[:, 65:66], erl[:, t:t + 1]
                )
                nc.sync.dma_start(send[t * 128:(t + 1) * 128, :], sends[t][:])
                if t == 3:
                    nc.gpsimd.collective_compute(
                        "AllGather", OP.bypass,
                        replica_groups=[list(range(NC))],
                        ins=[send[0:R // 2, :]], outs=[gathA[:]],
                    )
                    if li == 1:
                        load_thr_high()
            nc.gpsimd.collective_compute(
                "AllGather", OP.bypass,
                replica_groups=[list(range(NC))],
                ins=[send[R // 2:R, :]], outs=[gathB[:]],
            )

            # ---- under the gather: up/um, el broadcast ----
            up = sp.tile([128, NT], F32, tag=f"up{L}")
            nc.scalar.activation(up[:], el[:], AF.Exp)
            um = sp.tile([128, NT], F32, tag=f"um{L}")
            nc.scalar.activation(um[:], el[:], AF.Exp, scale=0.2)

            el_b = big.tile([128, R], BF16, tag="el_b", name=f"el_b{L}")
            el_dram = dram.tile([1, R], F32, tag=f"eld{L}")
            nc.sync.dma_start(
                el_dram[:].rearrange("one (t p) -> (one p) t", p=128), el[:]
            )
            el_row = sp.tile([1, R], F32, tag="el_row", name=f"el_row{L}")
            nc.sync.dma_start(el_row[:], el_dram[:])
            with tc.tile_pool(name=f"ps_e{L}", bufs=1, space="PSUM") as pse:
                ebp = pse.tile([128, R], F32, tag="ebp")
                for h in range(2):
                    nc.tensor.matmul(
                        ebp[:, h * 512:(h + 1) * 512],
                        ones1[:], el_row[:, h * 512:(h + 1) * 512],
                        start=True, stop=True,
                    )
                nc.vector.tensor_copy(el_b[:], ebp[:])

            # ---- masked attention loop (er/exp factors emitted per half) ----
            # acc[t] = [ A+@v+ | A+@v- (0:258) | thr@v- (258:387) ]; the spare
            # 387:512 of banks 0/1 accumulate CS = colsum(v-) from the SAME
            # bf16 v- values so the CS - thr@v-/256 - A+@v- cancellation is
            # exact at the bf16-input level.
            W2C = 2 * (D + 1)
            ones_bf = sp.tile([128, 1], BF16, tag="ones_bf", name=f"ones_bf{L}")
            nc.vector.memset(ones_bf[:], 1.0)
            psa = tc.alloc_tile_pool(name=f"ps_acc{L}", bufs=1, space="PSUM")
            acc = [
                psa.tile([128, 512], F32, tag=f"acc{t}", name=f"acc{t}_{L}")
                for t in range(NT)
            ]
            er_h = [None, None]
            wp_h = [None, None]
            wm_h = [None, None]
            xtg = None
            for k in range(NK):
                hi, kk = (0, k) if k < 32 else (1, k - 32)
                gath = gathA if hi == 0 else gathB
                if kk == 0:
                    er_x = sp.tile([128, 32], F32, tag=f"er{L}_{hi}")
                    nc.sync.dma_start(
                        er_x[:],
                        gath[:].bitcast(F32)[:, 65:66]
                        .rearrange("(c p) one -> p (c one)", p=128),
                    )
                    wp_x = sp.tile([128, 32], F32, tag=f"wp{L}_{hi}")
                    nc.scalar.activation(wp_x[:], er_x[:], AF.Exp)
                    wm_x = sp.tile([128, 32], F32, tag=f"wm{L}_{hi}")
                    nc.scalar.activation(wm_x[:], er_x[:], AF.Exp, scale=0.2)
                    er_h[hi], wp_h[hi], wm_h[hi] = er_x, wp_x, wm_x
                if kk % 8 == 0:
                    xtg = gpool.tile([128, 8, D + 1], BF16, tag="xtg",
                                     name=f"xtg{L}_{k // 8}")
                    nc.sync.dma_start(
                        xtg[:],
                        gath[(kk // 8) * 1024:(kk // 8 + 1) * 1024, 0:D + 1]
                        .rearrange("(c p) w -> p c w", p=128),
                    )
                j = kk % 8
                vch = vpool.tile([128, W2C], BF16, tag="vch", name=f"vch{L}_{k}")
                nc.vector.tensor_scalar(
                    vch[:, 0:D + 1], xtg[:, j, :], wp_h[hi][:, kk:kk + 1],
                    None, OP.mult,
                )
                nc.scalar.activation(
                    vch[:, D + 1:W2C], xtg[:, j, :], AF.Copy,
                    scale=wm_h[hi][:, kk:kk + 1],
                )
                ap_ = mpool.tile([128, R], BF16, tag="Ap")
                nc.vector.scalar_tensor_tensor(
                    ap_[:], el_b[:], er_h[hi][:, kk:kk + 1], thr_all[:, k, :],
                    OP.add, OP.is_gt,
                )
                first = k == 0
                last = k == NK - 1
                for t in range(NT):
                    nc.tensor.matmul(
                        acc[t][:, 0:W2C],
                        ap_[:, t * 128:(t + 1) * 128],
                        vch[:],
                        start=first, stop=False,
                    )
                    nc.tensor.matmul(
                        acc[t][:, W2C:W2C + D + 1],
                        thr_all[:, k, t * 128:(t + 1) * 128],
                        vch[:, D + 1:W2C],
                        start=False, stop=(last and t >= 2),
                    )
                nc.tensor.matmul(
                    acc[0][0:1, 387:512], ones_bf[:], vch[:, D + 1:D + 126],
                    start=False, stop=last,
                )
                nc.tensor.matmul(
                    acc[1][0:1, 387:391], ones_bf[:], vch[:, D + 126:W2C],
                    start=False, stop=last,
                )

            # ---- assemble + broadcast the colsum ----
            cs_row = sp.tile([1, D + 1], F32, tag="cs_row", name=f"cs_row{L}")
            nc.vector.tensor_copy(cs_row[:, 0:125], acc[0][0:1, 387:512])
            nc.vector.tensor_copy(cs_row[:, 125:129], acc[1][0:1, 387:391])
            csb = big.tile([128, D + 1], F32, tag="csb", name=f"csb{L}")
            nc.gpsimd.partition_broadcast(csb[:], cs_row[:])

            # ---- evict + normalize -> agg ----
            agg_tiles = []
            for t in range(NT):
                z = work.tile([128, D + 1], F32, tag="evz", bufs=2)
                nc.vector.scalar_tensor_tensor(
                    z[:], acc[t][:, W2C:W2C + D + 1], -1.0 / BIG, csb[:],
                    OP.mult, OP.add,
                )
                nc.vector.scalar_tensor_tensor(
                    z[:], acc[t][:, D + 1:W2C], -1.0, z[:], OP.mult, OP.add
                )
                r1 = work.tile([128, D + 1], F32, tag="evr", bufs=2)
                nc.vector.tensor_scalar(
                    r1[:], acc[t][:, 0:D + 1], up[:, t:t + 1], None, OP.mult
                )
                nc.vector.scalar_tensor_tensor(
                    r1[:], z[:], um[:, t:t + 1], r1[:], OP.mult, OP.add,
                )
                dn = sp.tile([128, 1], F32, tag="dn", bufs=2)
                nc.vector.tensor_scalar(dn[:], r1[:, D:D + 1], MIN, None, OP.max)
                idn = sp.tile([128, 1], F32, tag="idn", bufs=2)
                nc.vector.reciprocal(idn[:], dn[:])
                ag = work.tile([128, D], F32, tag=f"ag{L}_{t}")
                nc.scalar.activation(ag[:], r1[:, 0:D], AF.Copy, scale=idn[:])
                agg_tiles.append(ag)
            psa.release()

            if stop == 5 and li == 1:
                early_out(agg_tiles)
                return None

            # ---- tail: rf = relu(sc3*agg) in-place; next p = fsc'*rf ----
            an2 = sp.tile([128, NT], F32, tag=f"an2{L}")
            for t in range(NT):
                _dot_cols(nc, sp, an2[:, t:t + 1], agg_tiles[t], agg_tiles[t])
            an = sp.tile([128, NT], F32, tag=f"an{L}")
            _sqrt(nc, sp, an, an2, f"an_sq{L}")
            ath2 = sp.tile([128, NT], F32, tag=f"ath2{L}")
            _tanh(nc, sp, ath2, an, f"anth{L}")
            c3 = sp.tile([128, NT], F32, tag=f"c3{L}")
            nc.vector.tensor_scalar(c3[:], ath2[:], MAXN, ATANH_CLIP, OP.min, OP.min)
            at3 = sp.tile([128, NT], F32, tag=f"at3{L}")
            _atanh(nc, sp, at3, c3, f"at3{L}")
            ian = sp.tile([128, NT], F32, tag=f"ian{L}")
            nc.vector.reciprocal(ian[:], an[:])
            sc3 = sp.tile([128, NT], F32, tag=f"sc3{L}")
            nc.vector.tensor_tensor(sc3[:], at3[:], ian[:], OP.mult)
            rn2 = sp.tile([128, NT], F32, tag=f"rn2{L}")
            for t in range(NT):
                nc.vector.tensor_scalar(
                    agg_tiles[t][:], agg_tiles[t][:], sc3[:, t:t + 1], 0.0,
                    OP.mult, OP.max,
                )
                _dot_cols(nc, sp, rn2[:, t:t + 1], agg_tiles[t], agg_tiles[t])
            rn = sp.tile([128, NT], F32, tag=f"rn{L}")
            _sqrt(nc, sp, rn, rn2, f"rn_sq{L}")
            rth = sp.tile([128, NT], F32, tag=f"rth{L}")
            _tanh(nc, sp, rth, rn, f"rnth{L}")
            rthc = sp.tile([128, NT], F32, tag=f"rthc{L}")
            nc.vector.tensor_scalar(rthc[:], rth[:], MAXN, MIN, OP.min, OP.max)
            irn = sp.tile([128, NT], F32, tag=f"irn{L}")
            nc.vector.reciprocal(irn[:], rn[:])
            fsc2 = sp.tile([128, NT], F32, tag=f"fsc{L}")
            nc.vector.tensor_tensor(fsc2[:], rthc[:], irn[:], OP.mult)
            return agg_tiles, fsc2, rthc

        # ---------------- two layers + output ----------------
        if stop >= 3:
            res1 = layer(x_tiles, fsc_enc, xn_enc, 1)
            if res1 is not None:
                if stop == 6:
                    early_out(res1[0])
                else:
                    rf2, fscf, _ = layer(res1[0], res1[1], res1[2], 2)
                    for t in range(NT):
                        yt = work.tile([128, D], F32, tag="yt", bufs=2, name=f"yt{t}")
                        nc.scalar.activation(
                            yt[:], rf2[t][:], AF.Copy, scale=fscf[:, t:t + 1]
                        )
                        nc.sync.dma_start(y_out[t * 128:(t + 1) * 128, :], yt[:])

    nc.compile()
    return nc


_PROGRAM = None
_last_in_maps = None


def _host_prep(inputs):
    """Host-side constants: transposed weights, hyperbolic bias points, and
    the per-core threshold matrix in program-chunk order."""
    x = np.ascontiguousarray(inputs["x"], np.float32)
    adj = np.asarray(inputs["adj"], np.float32)
    base = {
        "WT1": np.ascontiguousarray(np.asarray(inputs["W1"]).T, np.float32),
        "WT2": np.ascontiguousarray(np.asarray(inputs["W2"]).T, np.float32),
        "ident_f32": np.eye(128, dtype=np.float32),
    }
    for li in (1, 2):
        b = np.asarray(inputs[f"b{li}"], np.float64).reshape(D)
        bn = max(np.linalg.norm(b), MIN)
        bh = np.tanh(bn) * b / bn
        bhn = np.linalg.norm(bh)
        if bhn > MAXN:
            bh = bh / bhn * MAXN
        base[f"bh{li}"] = bh.astype(np.float32).reshape(1, D)
        base[f"bh2_{li}"] = np.array([[float(np.dot(bh, bh))]], dtype=np.float32)
        base[f"al{li}"] = np.asarray(inputs[f"al{li}"], np.float32).reshape(1, D)
        base[f"ar{li}"] = np.asarray(inputs[f"ar{li}"], np.float32).reshape(1, D)

    po = np.asarray(PO)
    in_maps = []
    for r in range(NC):
        m = dict(base)
        m["x_shard"] = x[r * R:(r + 1) * R]
        thrT = (256.0 - 256.0 * adj[r * R:(r + 1) * R].T).astype(ml_dtypes.bfloat16)
        thr_po = thrT.reshape(NK, 128, R)[po]          # [k, p, i] program order
        m["thr_shard"] = np.ascontiguousarray(thr_po.reshape(N, R))
        in_maps.append(m)
    return in_maps


def kernel(**inputs):
    global _PROGRAM, _last_in_maps
    if _PROGRAM is None:
        _PROGRAM = build_program()
    nc = _PROGRAM
    in_maps = _host_prep(inputs)
    _last_in_maps = in_maps
    res = bass_utils.run_bass_kernel_spmd(nc, in_maps, core_ids=list(range(NC)))
    return np.concatenate([res.results[r]["y_shard"] for r in range(NC)], axis=0)


if __name__ == "__main__":
    import reference
    inputs = {k: np.asarray(v) for k, v in reference.setup_inputs().items()}
    out = kernel(**inputs)
    print("out", out.shape, out.dtype)


# revision 15
# speedup vs baseline: 1.3169x; 1.1214x over previous
"""HGAT (2-layer hyperbolic graph attention) Trainium2 kernel, 8-core SPMD.

Sharding: nodes (rows of x/adj) split 8 ways. Per layer the [N,132] payload
(xt bf16 | 1 | pad | er f32) is all-gathered in two row-halves so the mask
loop can start after the first half lands; softmax rows are local.

Attention decomposition (exact): with s_ij = el_i + er_j,
  exp(leaky_relu(s)) = 1{s>0} e^{el_i} e^{er_j} + 1{s<=0} e^{.2 el_i} e^{.2 er_j}
so  att-weighted agg = [u+ . (A+ @ v+) + u- . (A- @ v-)] row-normalized, where
  A+ = adj * 1{s>0} (bf16 0/1 mask vs a resident threshold thr = 256*(1-adj^T)
  host-prepped in processing-chunk order), and
  A-@v- = CS - (thr@v-)/256 - A+@v-  with CS = colsum(v-) accumulated in the
  spare PSUM columns of the acc banks from the SAME bf16 v- values, keeping
  the cancellation exact.
All hyperbolic chains (expmap/logmap/mobius ops) are folded into per-node
column scalars. Between stages the TANGENT NORM is propagated (at = atanh of
the clipped radius == min(norm, atanh(MAXN))), which eliminates every
tanh/atanh pair except one real tanh (expmap radius) and one real atanh
(logmap after mobius_add) per layer.
"""
import sys
import numpy as np

sys.path.insert(0, "/opt/trn_rl_repo")
sys.path.insert(0, "/opt/trn_rl_repo/concourse")

import ml_dtypes
from contextlib import ExitStack

import concourse.bass as bass
import concourse.tile as tile
from concourse import bacc, mybir
from concourse import bass_utils

F32 = mybir.dt.float32
BF16 = mybir.dt.bfloat16
AF = mybir.ActivationFunctionType
OP = mybir.AluOpType

N = 8192
D = 128
NC = 8
R = N // NC          # 1024 local rows
NT = R // 128        # 8 row tiles
NK = N // 128        # 64 j-chunks
MIN = 1e-15
MIN2 = 1e-30         # guard on squared norms (== MIN^2)
ATANH_CLIP = 1.0 - 1e-5
MAXN = 1.0 - 4e-3
ATH = 3.106303047875759   # atanh(MAXN)
BIG = 256.0
GW = 132             # gather row width (bf16): 128 xt + 1 one + 1 pad + 2 (er as f32)

# program chunk order: first the 32 "low half" chunks (each core's local rows
# 0:512), then the 32 "high half" chunks. PO[k] = global chunk index.
PO = [8 * r + hc for r in range(8) for hc in range(4)] + \
     [8 * r + hc for r in range(8) for hc in range(4, 8)]


def _atanh(nc, pool, out, c, tag):
    """out = atanh(c) = 0.5*ln((1+c)/(1-c)); c in [0, 1-4e-3]."""
    p, k = c.shape
    ap1 = pool.tile([p, k], F32, tag=tag + "_ap")
    nc.vector.tensor_scalar(ap1[:], c[:], 1.0, None, OP.add)
    am1 = pool.tile([p, k], F32, tag=tag + "_am")
    nc.vector.tensor_scalar(am1[:], c[:], -1.0, 1.0, OP.mult, OP.add)
    inv = pool.tile([p, k], F32, tag=tag + "_inv")
    nc.vector.reciprocal(inv[:], am1[:])
    ratio = pool.tile([p, k], F32, tag=tag + "_ratio")
    nc.vector.tensor_tensor(ratio[:], ap1[:], inv[:], OP.mult)
    ln = pool.tile([p, k], F32, tag=tag + "_lg")
    nc.scalar.activation(ln[:], ratio[:], AF.Ln)
    nc.vector.tensor_scalar(out[:], ln[:], 0.5, None, OP.mult)


def _norm_inv(nc, pool, nrm, inv, in2, tag):
    """nrm = sqrt(max(in2, MIN2)), inv = 1/nrm. Sqrt on ACT, recip on DVE."""
    p, k = in2.shape
    g = pool.tile([p, k], F32, tag=tag + "_g")
    nc.vector.tensor_scalar(g[:], in2[:], MIN2, None, OP.max)
    nc.scalar.activation(nrm[:], g[:], AF.Sqrt)
    nc.vector.reciprocal(inv[:], nrm[:])


def _dot_dve(nc, pool, out_col, a, b_t):
    """out_col [p,1] = sum over free dim of a*b_t, on DVE STT with accum."""
    p = a.shape[0]
    k = int(np.prod(a.shape[1:]))
    scratch = pool.tile([p, k], F32, tag="dot_scr", bufs=4)
    nc.vector.scalar_tensor_tensor(
        scratch[:], a[:], 1.0, b_t[:], OP.mult, OP.mult, accum_out=out_col[:]
    )


def _dot_self_act(nc, pool, out_col, a):
    """out_col [p,1] = sum(a*a) on ACT (Square + accumulate)."""
    p = a.shape[0]
    k = int(np.prod(a.shape[1:]))
    scratch = pool.tile([p, k], F32, tag="dot_scr2", bufs=4)
    nc.scalar.activation(scratch[:], a[:], AF.Square, accum_out=out_col[:])


def build_program():
    import os
    stop = int(os.environ.get("HGAT_STOP", "9"))
    nc = bacc.Bacc(
        "TRN2", target_bir_lowering=False, debug=False, num_devices=NC
    )
    try:
        from concourse import tile_utils
        tile_utils.max_sbuf_usage = 206 * 1024
    except Exception:
        pass
    try:
        tile.max_sbuf_usage = 206 * 1024
    except Exception:
        pass

    x_in = nc.dram_tensor("x_shard", [R, D], F32, kind="ExternalInput").ap()
    # thr in PROGRAM chunk order: [N, R] rows grouped as (k, p)
    thr_in = nc.dram_tensor("thr_shard", [N, R], BF16, kind="ExternalInput").ap()
    wts = {}
    for li in (1, 2):
        wts[f"WT{li}"] = nc.dram_tensor(f"WT{li}", [D, D], F32, kind="ExternalInput").ap()
        for v in ("al", "ar", "bh"):
            wts[f"{v}{li}"] = nc.dram_tensor(f"{v}{li}", [1, D], F32, kind="ExternalInput").ap()
        wts[f"bh2_{li}"] = nc.dram_tensor(f"bh2_{li}", [1, 1], F32, kind="ExternalInput").ap()
    ident_f32 = nc.dram_tensor("ident_f32", [128, 128], F32, kind="ExternalInput").ap()
    y_out = nc.dram_tensor("y_shard", [R, D], F32, kind="ExternalOutput").ap()

    with tile.TileContext(nc) as tc, ExitStack() as ctx:
        # ---------------- pools ----------------
        big = ctx.enter_context(tc.tile_pool(name="big", bufs=1))      # residents
        sp = ctx.enter_context(tc.tile_pool(name="scal", bufs=1))      # small scalar tiles
        work = ctx.enter_context(tc.tile_pool(name="work", bufs=1))    # [128,128] f32 named tiles
        stage = ctx.enter_context(tc.tile_pool(name="stage", bufs=1))
        mpool = ctx.enter_context(tc.tile_pool(name="masks", bufs=2))
        spool = ctx.enter_context(tc.tile_pool(name="scores", bufs=2))
        vpool = ctx.enter_context(tc.tile_pool(name="vchunk", bufs=3))
        gpool = ctx.enter_context(tc.tile_pool(name="gblk", bufs=3))
        dram = ctx.enter_context(tc.tile_pool(name="dram", bufs=1, space="DRAM"))

        # ---------------- early small loads (sync queue, first) --------------
        idf = big.tile([128, 128], F32, tag="idf")
        nc.sync.dma_start(idf[:], ident_f32[:])
        x_tiles = []
        for t in range(NT):
            xt_ = work.tile([128, D], F32, tag=f"x{t}")
            nc.sync.dma_start(xt_[:], x_in[t * 128:(t + 1) * 128, :])
            x_tiles.append(xt_)
        w_sb = {}
        for li in (1, 2):
            w_sb[f"WT{li}"] = big.tile([128, 128], F32, tag=f"WT{li}",
                                       name=f"WT{li}")
            nc.sync.dma_start(w_sb[f"WT{li}"][:], wts[f"WT{li}"][:])
            for v in ("al", "ar", "bh"):
                w_sb[f"{v}{li}"] = sp.tile([1, D], F32, tag=f"{v}{li}_sb",
                                           name=f"{v}{li}_sb")
                nc.sync.dma_start(w_sb[f"{v}{li}"][:], wts[f"{v}{li}"][:])
            w_sb[f"bh2_{li}"] = sp.tile([1, 1], F32, tag=f"bh2_{li}_sb",
                                        name=f"bh2_{li}_sb")
            nc.sync.dma_start(w_sb[f"bh2_{li}"][:], wts[f"bh2_{li}"][:])

        # ------- thr resident, loaded on the gpsimd queue, low half first ----
        # thr_all[p, k, i] = 256*(1 - adj[i_global, jnode(PO[k], p)])
        thr_all = big.tile([128, NK, R], BF16, tag="thr")
        for q in range(4):  # low half: program chunks 0..31
            nc.gpsimd.dma_start(
                thr_all[:, q * 8:(q + 1) * 8, :],
                thr_in[q * 8 * 128:(q + 1) * 8 * 128, :]
                .rearrange("(c p) i -> p c i", p=128),
            )

        def load_thr_high():
            for q in range(4, 8):  # high half: program chunks 32..63
                nc.gpsimd.dma_start(
                    thr_all[:, q * 8:(q + 1) * 8, :],
                    thr_in[q * 8 * 128:(q + 1) * 8 * 128, :]
                    .rearrange("(c p) i -> p c i", p=128),
                )

        # ---------------- broadcast constants via K=1 matmuls ----------------
        ones1 = sp.tile([1, 128], F32, tag="ones1")
        nc.vector.memset(ones1[:], 1.0)
        bcast = {}
        with tc.tile_pool(name="ps_b", bufs=2, space="PSUM") as psb:
            for li in (1, 2):
                for v in ("al", "ar", "bh"):
                    ps = psb.tile([128, 128], F32, tag="bc_ps")
                    nc.tensor.matmul(ps[:], ones1[:], w_sb[f"{v}{li}"][:],
                                     start=True, stop=True)
                    bb = big.tile([128, 128], F32, tag=f"{v}{li}_b",
                                  name=f"{v}{li}_b")
                    nc.vector.tensor_copy(bb[:], ps[:])
                    bcast[f"{v}{li}"] = bb
                ps1 = psb.tile([128, 1], F32, tag="bc_ps1")
                nc.tensor.matmul(ps1[:], ones1[:], w_sb[f"bh2_{li}"][:],
                                 start=True, stop=True)
                b1 = sp.tile([128, 1], F32, tag=f"bh2_{li}_b",
                             name=f"bh2_{li}_b")
                nc.vector.tensor_copy(b1[:], ps1[:])
                bcast[f"bh2_{li}"] = b1

        def early_out(tiles, width=D):
            for tt_, tl in enumerate(tiles):
                cp = work.tile([128, D], F32, tag="eo", bufs=2, name=f"eo{tt_}")
                nc.vector.tensor_copy(cp[:], tl[:] if tl.shape[-1] == width else tl[:, 0:width])
                nc.sync.dma_start(y_out[tt_ * 128:(tt_ + 1) * 128, :], cp[:])

        # -------- encode (folded): p = (at/rn applied later); rn = |x| -------
        xn2 = sp.tile([128, NT], F32, tag="enc_n2")
        for t in range(NT):
            _dot_self_act(nc, sp, xn2[:, t:t + 1], x_tiles[t])
        ixn_enc = sp.tile([128, NT], F32, tag="enc_ixn")
        xnr = sp.tile([128, NT], F32, tag="enc_nr")
        _norm_inv(nc, sp, xnr, ixn_enc, xn2, "enc")
        at_enc = sp.tile([128, NT], F32, tag="enc_at")
        nc.vector.tensor_scalar(at_enc[:], xnr[:], ATH, None, OP.min)

        if stop == 2:
            early_out(x_tiles)

        # ---------------- layer ----------------
        def layer(ft, irn_in, at_in, li):
            """Input: hyperbolic p = tanh(at_in)*irn_in*ft (radius clipped),
            |p| = tanh(at_in) <= MAXN, at_in = atanh(|p|), irn_in = 1/|ft|.
            xt overwrites ft; returns (rf, irn', at') for the next layer."""
            L = f"l{li}"
            al_b, ar_b, bh_b = bcast[f"al{li}"], bcast[f"ar{li}"], bcast[f"bh{li}"]
            bh2_b = bcast[f"bh2_{li}"]
            WT_sb = w_sb[f"WT{li}"]

            # at*irn is input-derived; compute before fw lands
            ta = sp.tile([128, NT], F32, tag=f"ta{L}")
            nc.vector.tensor_tensor(ta[:], at_in[:], irn_in[:], OP.mult)

            # ---- matvec: fw = ft @ WT ----
            fw = []
            with tc.tile_pool(name=f"ps_w{L}", bufs=2, space="PSUM") as psw:
                for t in range(NT):
                    ptp = psw.tile([128, 128], F32, tag="ptp")
                    nc.tensor.transpose(ptp[:], ft[t][:], idf[:])
                    pT = work.tile([128, 128], F32, tag="pT", bufs=2)
                    nc.vector.tensor_copy(pT[:], ptp[:])
                    mxp = psw.tile([128, 128], F32, tag="mxp")
                    nc.tensor.matmul(mxp[:], pT[:], WT_sb[:], start=True, stop=True)
                    fw_t = work.tile([128, 128], F32, tag=f"fw{t}",
                                     name=f"fw{L}_{t}")
                    nc.vector.tensor_copy(fw_t[:], mxp[:])
                    fw.append(fw_t)

            # self-dots on ACT, bh-dots on DVE — they run in parallel
            fwn2 = sp.tile([128, NT], F32, tag=f"fwn2{L}")
            xyr = sp.tile([128, NT], F32, tag=f"xyr{L}")
            for t in range(NT):
                _dot_self_act(nc, sp, fwn2[:, t:t + 1], fw[t])
                _dot_dve(nc, sp, xyr[:, t:t + 1], fw[t], bh_b)
            ifwn = sp.tile([128, NT], F32, tag=f"ifwn{L}")
            fwn = sp.tile([128, NT], F32, tag=f"fwn{L}")
            _norm_inv(nc, sp, fwn, ifwn, fwn2, f"fw{L}")

            # ---- mobius_matvec scalars: h = lam*fw, |h| = thmc ----
            arg = sp.tile([128, NT], F32, tag=f"arg{L}")
            nc.vector.tensor_tensor(arg[:], ta[:], fwn[:], OP.mult)
            thm = sp.tile([128, NT], F32, tag=f"thm{L}")
            nc.scalar.activation(thm[:], arg[:], AF.Tanh)
            thmc = sp.tile([128, NT], F32, tag=f"thmc{L}")
            nc.vector.tensor_scalar(thmc[:], thm[:], MAXN, None, OP.min)
            lam = sp.tile([128, NT], F32, tag=f"lam{L}")
            nc.vector.tensor_tensor(lam[:], thmc[:], ifwn[:], OP.mult)

            # ---- mobius_add + proj + logmap0 folded: xt = a'*fw + bet*bh ----
            x2 = sp.tile([128, NT], F32, tag=f"x2{L}")
            nc.vector.tensor_tensor(x2[:], thmc[:], thmc[:], OP.mult)
            xy = sp.tile([128, NT], F32, tag=f"xy{L}")
            nc.vector.tensor_tensor(xy[:], lam[:], xyr[:], OP.mult)
            # cf = 1 + 2xy + y2 ; cb = 1 - x2 ; den = max(1+2xy+x2*y2, MIN)
            cf = sp.tile([128, NT], F32, tag=f"cf{L}")
            nc.vector.tensor_scalar(cf[:], xy[:], 2.0, 1.0, OP.mult, OP.add)
            nc.vector.tensor_scalar(cf[:], cf[:], bh2_b[:], None, OP.add)
            cb = sp.tile([128, NT], F32, tag=f"cb{L}")
            nc.vector.tensor_scalar(cb[:], x2[:], -1.0, 1.0, OP.mult, OP.add)
            x2y2 = sp.tile([128, NT], F32, tag=f"x2y2{L}")
            nc.vector.tensor_scalar(x2y2[:], x2[:], bh2_b[:], None, OP.mult)
            den = sp.tile([128, NT], F32, tag=f"den{L}")
            nc.vector.scalar_tensor_tensor(den[:], xy[:], 2.0, x2y2[:], OP.mult, OP.add)
            nc.vector.tensor_scalar(den[:], den[:], 1.0, MIN, OP.add, OP.max)
            # nn2 = cf^2 x2 + 2 cf cb xy + cb^2 y2 = |num|^2
            t1 = sp.tile([128, NT], F32, tag=f"t1{L}")
            nc.vector.tensor_tensor(t1[:], cf[:], cf[:], OP.mult)
            nc.vector.tensor_tensor(t1[:], t1[:], x2[:], OP.mult)
            t2 = sp.tile([128, NT], F32, tag=f"t2{L}")
            nc.vector.tensor_tensor(t2[:], cf[:], cb[:], OP.mult)
            nc.vector.tensor_tensor(t2[:], t2[:], xy[:], OP.mult)
            nc.vector.scalar_tensor_tensor(t1[:], t2[:], 2.0, t1[:], OP.mult, OP.add)
            t3 = sp.tile([128, NT], F32, tag=f"t3{L}")
            nc.vector.tensor_tensor(t3[:], cb[:], cb[:], OP.mult)
            nc.vector.tensor_scalar(t3[:], t3[:], bh2_b[:], None, OP.mult)
            nn2 = sp.tile([128, NT], F32, tag=f"nn2{L}")
            nc.vector.tensor_tensor(nn2[:], t1[:], t3[:], OP.add)
            inn = sp.tile([128, NT], F32, tag=f"inn{L}")
            nn = sp.tile([128, NT], F32, tag=f"nn{L}")
            _norm_inv(nc, sp, nn, inn, nn2, f"nn{L}")
            iden = sp.tile([128, NT], F32, tag=f"iden{L}")
            nc.vector.reciprocal(iden[:], den[:])
            hn = sp.tile([128, NT], F32, tag=f"hn{L}")
            nc.vector.tensor_tensor(hn[:], nn[:], iden[:], OP.mult)
            # logmap0(proj(h2)): psi = atanh(min(hn, MAXN)) / hn / den
            hmc = sp.tile([128, NT], F32, tag=f"hmc{L}")
            nc.vector.tensor_scalar(hmc[:], hn[:], MAXN, None, OP.min)
            ath = sp.tile([128, NT], F32, tag=f"ath{L}")
            _atanh(nc, sp, ath, hmc, f"ath{L}")
            ihn = sp.tile([128, NT], F32, tag=f"ihn{L}")
            nc.vector.reciprocal(ihn[:], hn[:])
            psi = sp.tile([128, NT], F32, tag=f"psi{L}")
            nc.vector.tensor_tensor(psi[:], ath[:], ihn[:], OP.mult)
            nc.vector.tensor_tensor(psi[:], psi[:], iden[:], OP.mult)
            alp = sp.tile([128, NT], F32, tag=f"alp{L}")
            nc.vector.tensor_tensor(alp[:], psi[:], cf[:], OP.mult)
            nc.vector.tensor_tensor(alp[:], alp[:], lam[:], OP.mult)
            bet = sp.tile([128, NT], F32, tag=f"bet{L}")
            nc.vector.tensor_tensor(bet[:], psi[:], cb[:], OP.mult)

            xt = ft  # xt overwrites the input tiles (ft only read by transpose)
            for t in range(NT):
                tmp = work.tile([128, 128], F32, tag="xtmp", bufs=2)
                nc.vector.tensor_scalar(tmp[:], fw[t][:], alp[:, t:t + 1], None, OP.mult)
                nc.vector.scalar_tensor_tensor(
                    xt[t][:], bh_b[:], bet[:, t:t + 1], tmp[:], OP.mult, OP.add
                )

            if stop == 3 and li == 1:
                early_out(xt)
                return None

            # ---- el, er_local ----
            el = sp.tile([128, NT], F32, tag=f"el{L}")
            erl = sp.tile([128, NT], F32, tag=f"erl{L}")
            for t in range(NT):
                _dot_dve(nc, sp, el[:, t:t + 1], xt[t], al_b)
                _dot_dve(nc, sp, erl[:, t:t + 1], xt[t], ar_b)

            # ---- send build + split AllGather ----
            send = dram.tile([R, GW], BF16, tag=f"send{L}")
            gathA = dram.tile([N // 2, GW], BF16, tag=f"gathA{L}", addr_space="Shared")
            gathB = dram.tile([N // 2, GW], BF16, tag=f"gathB{L}", addr_space="Shared")
            sends = []
            for t in range(NT):
                sb_send = stage.tile([128, GW], BF16, tag=f"sb_send{t}",
                                     name=f"sb_send{L}_{t}")
                sends.append(sb_send)
                nc.vector.memset(sb_send[:, D:D + 2], 0.0)
                nc.vector.memset(sb_send[:, D:D + 1], 1.0)
            for t in range(NT):
                nc.vector.tensor_copy(sends[t][:, 0:D], xt[t][:])
                nc.vector.tensor_copy(
                    sends[t][:].bitcast(F32)[:, 65:66], erl[:, t:t + 1]
                )
                nc.sync.dma_start(send[t * 128:(t + 1) * 128, :], sends[t][:])
                if t == 3:
                    nc.gpsimd.collective_compute(
                        "AllGather", OP.bypass,
                        replica_groups=[list(range(NC))],
                        ins=[send[0:R // 2, :]], outs=[gathA[:]],
                    )
                    if li == 1:
                        load_thr_high()
            nc.gpsimd.collective_compute(
                "AllGather", OP.bypass,
                replica_groups=[list(range(NC))],
                ins=[send[R // 2:R, :]], outs=[gathB[:]],
            )

            # ---- under the gather: up/um, el broadcast ----
            up = sp.tile([128, NT], F32, tag=f"up{L}")
            nc.scalar.activation(up[:], el[:], AF.Exp)
            um = sp.tile([128, NT], F32, tag=f"um{L}")
            nc.scalar.activation(um[:], el[:], AF.Exp, scale=0.2)

            el_b = big.tile([128, R], BF16, tag="el_b", name=f"el_b{L}")
            el_dram = dram.tile([1, R], F32, tag=f"eld{L}")
            nc.sync.dma_start(
                el_dram[:].rearrange("one (t p) -> (one p) t", p=128), el[:]
            )
            el_row = sp.tile([1, R], F32, tag="el_row", name=f"el_row{L}")
            nc.sync.dma_start(el_row[:], el_dram[:])
            with tc.tile_pool(name=f"ps_e{L}", bufs=1, space="PSUM") as pse:
                ebp = pse.tile([128, R], F32, tag="ebp")
                for h in range(2):
                    nc.tensor.matmul(
                        ebp[:, h * 512:(h + 1) * 512],
                        ones1[:], el_row[:, h * 512:(h + 1) * 512],
                        start=True, stop=True,
                    )
                nc.vector.tensor_copy(el_b[:], ebp[:])

            # ---- masked attention loop ----
            # acc[t] = [ A+@v+ | A+@v- (0:258) | thr@v- (258:387) ]; spare
            # 387:512 of banks 0/1 accumulate CS = colsum(v-) from the SAME
            # bf16 v- values so the CS-based cancellation is exact.
            W2C = 2 * (D + 1)
            ones_bf = sp.tile([128, 1], BF16, tag="ones_bf", name=f"ones_bf{L}")
            nc.vector.memset(ones_bf[:], 1.0)
            psa = tc.alloc_tile_pool(name=f"ps_acc{L}", bufs=1, space="PSUM")
            acc = [
                psa.tile([128, 512], F32, tag=f"acc{t}", name=f"acc{t}_{L}")
                for t in range(NT)
            ]
            er_h = []
            wp_h = []
            wm_h = []
            for hi in range(2):
                er_h.append(sp.tile([128, 32], F32, tag=f"er_{hi}",
                                    name=f"er{L}_{hi}"))
                wp_h.append(sp.tile([128, 32], F32, tag=f"wp_{hi}",
                                    name=f"wp{L}_{hi}"))
                wm_h.append(sp.tile([128, 32], F32, tag=f"wm_{hi}",
                                    name=f"wm{L}_{hi}"))

            def load_half(hi, gath):
                """Load one gather half as two 16-chunk blocks (contiguous
                full-width rows), extract er, compute exp factors."""
                blocks = []
                for b in range(2):
                    xg = gpool.tile([128, 16, GW], BF16, tag="xtg",
                                    name=f"xtg{L}_{hi}_{b}")
                    nc.sync.dma_start(
                        xg[:],
                        gath[b * 2048:(b + 1) * 2048, :]
                        .rearrange("(c p) w -> p c w", p=128),
                    )
                    nc.vector.tensor_copy(
                        er_h[hi][:, b * 16:(b + 1) * 16],
                        xg[:].bitcast(F32)[:, :, 65:66],
                    )
                    blocks.append(xg)
                nc.scalar.activation(wp_h[hi][:], er_h[hi][:], AF.Exp)
                nc.scalar.activation(wm_h[hi][:], er_h[hi][:], AF.Exp, scale=0.2)
                return blocks

            blocks = [load_half(0, gathA), None]
            for k in range(NK):
                hi, kk = (0, k) if k < 32 else (1, k - 32)
                if k == 24:  # prefetch half B while A's tail still computes
                    blocks[1] = load_half(1, gathB)
                xtg = blocks[hi][kk // 16]
                j = kk % 16
                vch = vpool.tile([128, W2C], BF16, tag="vch", name=f"vch{L}_{k}")
                nc.scalar.activation(
                    vch[:, 0:D + 1], xtg[:, j, 0:D + 1], AF.Copy,
                    scale=wp_h[hi][:, kk:kk + 1],
                )
                nc.scalar.activation(
                    vch[:, D + 1:W2C], xtg[:, j, 0:D + 1], AF.Copy,
                    scale=wm_h[hi][:, kk:kk + 1],
                )
                s_c = spool.tile([128, R], BF16, tag="sc")
                nc.vector.tensor_scalar(
                    s_c[:], el_b[:], er_h[hi][:, kk:kk + 1], None, OP.add
                )
                ap_ = mpool.tile([128, R], BF16, tag="Ap")
                nc.vector.tensor_tensor(ap_[:], s_c[:], thr_all[:, k, :], OP.is_gt)
                first = k == 0
                last = k == NK - 1
                for t in range(NT):
                    nc.tensor.matmul(
                        acc[t][:, 0:W2C],
                        ap_[:, t * 128:(t + 1) * 128],
                        vch[:],
                        start=first, stop=False,
                    )
                    nc.tensor.matmul(
                        acc[t][:, W2C:W2C + D + 1],
                        thr_all[:, k, t * 128:(t + 1) * 128],
                        vch[:, D + 1:W2C],
                        start=False, stop=(last and t >= 2),
                    )
                nc.tensor.matmul(
                    acc[0][0:1, 387:512], ones_bf[:], vch[:, D + 1:D + 126],
                    start=False, stop=last,
                )
                nc.tensor.matmul(
                    acc[1][0:1, 387:391], ones_bf[:], vch[:, D + 126:W2C],
                    start=False, stop=last,
                )

            # ---- assemble + broadcast the colsum ----
            cs_row = sp.tile([1, D + 1], F32, tag="cs_row", name=f"cs_row{L}")
            nc.vector.tensor_copy(cs_row[:, 0:125], acc[0][0:1, 387:512])
            nc.vector.tensor_copy(cs_row[:, 125:129], acc[1][0:1, 387:391])
            csb = big.tile([128, D + 1], F32, tag="csb", name=f"csb{L}")
            nc.gpsimd.partition_broadcast(csb[:], cs_row[:])

            # ---- evict + normalize -> agg ----
            agg_tiles = []
            for t in range(NT):
                z = work.tile([128, D + 1], F32, tag="evz", bufs=2)
                nc.vector.scalar_tensor_tensor(
                    z[:], acc[t][:, W2C:W2C + D + 1], -1.0 / BIG, csb[:],
                    OP.mult, OP.add,
                )
                nc.vector.scalar_tensor_tensor(
                    z[:], acc[t][:, D + 1:W2C], -1.0, z[:], OP.mult, OP.add
                )
                r1 = work.tile([128, D + 1], F32, tag="evr", bufs=2)
                nc.vector.tensor_scalar(
                    r1[:], acc[t][:, 0:D + 1], up[:, t:t + 1], None, OP.mult
                )
                nc.vector.scalar_tensor_tensor(
                    r1[:], z[:], um[:, t:t + 1], r1[:], OP.mult, OP.add,
                )
                dn = sp.tile([128, 1], F32, tag="dn", bufs=2)
                nc.vector.tensor_scalar(dn[:], r1[:, D:D + 1], MIN, None, OP.max)
                idn = sp.tile([128, 1], F32, tag="idn", bufs=2)
                nc.vector.reciprocal(idn[:], dn[:])
                ag = work.tile([128, D], F32, tag=f"ag{L}_{t}")
                nc.scalar.activation(ag[:], r1[:, 0:D], AF.Copy, scale=idn[:])
                agg_tiles.append(ag)
            psa.release()

            if stop == 5 and li == 1:
                early_out(agg_tiles)
                return None

            # ---- tail: rf = relu(sc3*agg) in-place; sc3 = min(an,ATH)/an ----
            an2 = sp.tile([128, NT], F32, tag=f"an2{L}")
            for t in range(NT):
                _dot_self_act(nc, sp, an2[:, t:t + 1], agg_tiles[t])
            ian = sp.tile([128, NT], F32, tag=f"ian{L}")
            an = sp.tile([128, NT], F32, tag=f"an{L}")
            _norm_inv(nc, sp, an, ian, an2, f"an{L}")
            at3 = sp.tile([128, NT], F32, tag=f"at3{L}")
            nc.vector.tensor_scalar(at3[:], an[:], ATH, None, OP.min)
            sc3 = sp.tile([128, NT], F32, tag=f"sc3{L}")
            nc.vector.tensor_tensor(sc3[:], at3[:], ian[:], OP.mult)
            rn2 = sp.tile([128, NT], F32, tag=f"rn2{L}")
            for t in range(NT):
                nc.vector.tensor_scalar(
                    agg_tiles[t][:], agg_tiles[t][:], sc3[:, t:t + 1], 0.0,
                    OP.mult, OP.max,
                )
                _dot_self_act(nc, sp, rn2[:, t:t + 1], agg_tiles[t])
            irn2 = sp.tile([128, NT], F32, tag=f"irn{L}")
            rn = sp.tile([128, NT], F32, tag=f"rn{L}")
            _norm_inv(nc, sp, rn, irn2, rn2, f"rn{L}")
            at2 = sp.tile([128, NT], F32, tag=f"atn{L}")
            nc.vector.tensor_scalar(at2[:], rn[:], ATH, None, OP.min)
            return agg_tiles, irn2, at2, rn

        # ---------------- two layers + output ----------------
        if stop >= 3:
            res1 = layer(x_tiles, ixn_enc, at_enc, 1)
            if res1 is not None:
                if stop == 6:
                    early_out(res1[0])
                else:
                    rf2, irn_f, _, rn_f = layer(res1[0], res1[1], res1[2], 2)
                    # final output scale: min(tanh(rn), MAXN)/rn, guarded
                    rthf = sp.tile([128, NT], F32, tag="rthf")
                    nc.scalar.activation(rthf[:], rn_f[:], AF.Tanh)
                    nc.vector.tensor_scalar(rthf[:], rthf[:], MAXN, MIN,
                                            OP.min, OP.max)
                    fscf = sp.tile([128, NT], F32, tag="fscf")
                    nc.vector.tensor_tensor(fscf[:], rthf[:], irn_f[:], OP.mult)
                    for t in range(NT):
                        yt = work.tile([128, D], F32, tag="yt", bufs=2, name=f"yt{t}")
                        nc.scalar.activation(
                            yt[:], rf2[t][:], AF.Copy, scale=fscf[:, t:t + 1]
                        )
                        nc.sync.dma_start(y_out[t * 128:(t + 1) * 128, :], yt[:])

    nc.compile()
    return nc


_PROGRAM = None
_last_in_maps = None


def _host_prep(inputs):
    """Host-side constants: transposed weights, hyperbolic bias points, and
    the per-core threshold matrix in program-chunk order."""
    x = np.ascontiguousarray(inputs["x"], np.float32)
    adj = np.asarray(inputs["adj"], np.float32)
    base = {
        "WT1": np.ascontiguousarray(np.asarray(inputs["W1"]).T, np.float32),
        "WT2": np.ascontiguousarray(np.asarray(inputs["W2"]).T, np.float32),
        "ident_f32": np.eye(128, dtype=np.float32),
    }
    for li in (1, 2):
        b = np.asarray(inputs[f"b{li}"], np.float64).reshape(D)
        bn = max(np.linalg.norm(b), MIN)
        bh = np.tanh(bn) * b / bn
        bhn = np.linalg.norm(bh)
        if bhn > MAXN:
            bh = bh / bhn * MAXN
        base[f"bh{li}"] = bh.astype(np.float32).reshape(1, D)
        base[f"bh2_{li}"] = np.array([[float(np.dot(bh, bh))]], dtype=np.float32)
        base[f"al{li}"] = np.asarray(inputs[f"al{li}"], np.float32).reshape(1, D)
        base[f"ar{li}"] = np.asarray(inputs[f"ar{li}"], np.float32).reshape(1, D)

    po = np.asarray(PO)
    in_maps = []
    for r in range(NC):
        m = dict(base)
        m["x_shard"] = x[r * R:(r + 1) * R]
        thrT = (256.0 - 256.0 * adj[r * R:(r + 1) * R].T).astype(ml_dtypes.bfloat16)
        thr_po = thrT.reshape(NK, 128, R)[po]          # [k, p, i] program order
        m["thr_shard"] = np.ascontiguousarray(thr_po.reshape(N, R))
        in_maps.append(m)
    return in_maps


def kernel(**inputs):
    global _PROGRAM, _last_in_maps
    if _PROGRAM is None:
        _PROGRAM = build_program()
    nc = _PROGRAM
    in_maps = _host_prep(inputs)
    _last_in_maps = in_maps
    res = bass_utils.run_bass_kernel_spmd(nc, in_maps, core_ids=list(range(NC)))
    return np.concatenate([res.results[r]["y_shard"] for r in range(NC)], axis=0)


if __name__ == "__main__":
    import reference
    inputs = {k: np.asarray(v) for k, v in reference.setup_inputs().items()}
    out = kernel(**inputs)
    print("out", out.shape, out.dtype)
